# revision 41
# baseline (speedup 1.0000x reference)
"""Trainium2 Bass kernel for per-sample expert-routed 2-layer MLP (MoE routing).

Problem: logits[b] = relu(V[b] @ W1[id[b]] + b1[id[b]]) @ W2[id[b]] + b2[id[b]]
  V = concat(v_X, v_H): (256, 1536), 32 experts, W1 per expert (1536, 768).

Strategy (expert parallel over 8 NeuronCores, raw bacc pipeline):
  - Host routes samples to experts, assigns 4 experts per core, and casts
    W1 to float8_e3m4 (x128 scale, folded out of b1/W2 on host), so each
    expert's 1.18 MB streams HBM->SBUF exactly once chip-wide at one byte
    per element -- half the bf16 traffic.  The whole per-core W1 (4.7 MB
    = 36.9 KB/partition) is SBUF-resident: no ring, no recycle waits.
  - W1 slab DMAs issue from the sync engine (fine-grained for expert 0 so
    the PE starts early, 6-k-tile slabs after -- dma_start issue costs
    ~750ns + 5ns/descriptor of sequencer time, so big slabs keep the
    stream issue-rate above the queue bandwidth).  vt/b1/w2 issue from
    GpSimd, whose Q7 boots earliest.
  - The TensorEngine chases the stream with V^T stationary (bf16; matmul
    operands mix dtypes in normal mode).  PSUM regions 0:512/512:768
    alternate per k-tile: consecutive matmuls into the same PSUM bank
    stall ~330ns, alternating banks run back-to-back.  A dummy-matmul
    burst at boot starts the PE clock ramp before real data lands.
  - Bias starts each PSUM accumulation group as a K=1 matmul against an
    all-ones row.
  - Layer 2 (768 -> 2): four fused multiply-reduce passes on the DVE
    (tensor_tensor_reduce, region-chained via the init-value AP; b2 via a
    ones column), overlapped with the scalar engine's two-region relu.
  - Outputs (capacity-padded per-expert logits) are scattered on host.
"""

from contextlib import ExitStack

import ml_dtypes
import numpy as np

import concourse.bacc as bacc
import concourse.mybir as mybir
from concourse.bass_utils import run_bass_kernel_spmd

N_CORES = 8
KT = 12          # K tiles of 128 over D=1536
D = 1536
H = 768
W1_SCALE = 128.0
N_WARMUP_512 = 9
N_WARMUP_256 = 1
# k-tiles per W1 slab DMA, per expert-group.  Every slab costs 128
# descriptors (~0.1-0.25us of queue processing each, plus ~0.8us of
# sequencer issue), so k-tile SUPPLY RATE scales with slab size: the
# head matches the mid-clock PE rate, late experts ship whole to cut
# total descriptor count (and so total queue time).
K_CHUNKS = ((1, 2, 3, 3, 3), (6, 6), (6, 6), (6, 6))

_graph_cache = {}


def _build(G: int, C: int):
    """Build the SPMD graph: G expert-groups per core, capacity C samples."""
    dt = mybir.dt
    Act = mybir.ActivationFunctionType
    Alu = mybir.AluOpType

    nc = bacc.Bacc("TRN2", target_bir_lowering=False, debug=False,
                   enable_asserts=False, monotonic_sem_count=0)

    chunks = []  # (global_id, g, k0, kc)
    for g in range(G):
        for kc in K_CHUNKS[g] if g < len(K_CHUNKS) else K_CHUNKS[-1]:
            k0 = sum(c[3] for c in chunks if c[1] == g)
            chunks.append((len(chunks), g, k0, kc))
    NSLAB = len(chunks)

    w1d = nc.dram_tensor("w1", [G, 128, KT, H], dt.float8e3, kind="ExternalInput")
    vtd = nc.dram_tensor("vt", [128, KT + 1, G, C], dt.bfloat16, kind="ExternalInput")
    b1d = nc.dram_tensor("b1r", [1, G, H], dt.bfloat16, kind="ExternalInput")
    w2d = nc.dram_tensor("w2e", [C, G, 2, H + 1], dt.bfloat16, kind="ExternalInput")
    outd = nc.dram_tensor("out", [C, 2 * G], dt.float32, kind="ExternalOutput")

    regions = ((0, 512), (512, H))

    with ExitStack() as ctx:
        en = ctx.enter_context
        wz = en(nc.sbuf_tensor("wz", [128, 512], dt.bfloat16))
        w1 = en(nc.sbuf_tensor("w1_sb", [128, G, KT, H], dt.float8e3))
        vt = en(nc.sbuf_tensor("vt_sb", [128, KT + 1, G, C], dt.bfloat16))
        b1 = en(nc.sbuf_tensor("b1_sb", [1, G, H], dt.bfloat16))
        w2 = en(nc.sbuf_tensor("w2_sb", [C, G, 2, H + 1], dt.bfloat16))
        o1 = en(nc.sbuf_tensor("o1", [C, 2, H + 1], dt.bfloat16))
        scr = en(nc.sbuf_tensor("scr", [C, 2, H + 1], dt.bfloat16))
        logits = en(nc.sbuf_tensor("logits", [C, 2 * G], dt.float32))
        part = en(nc.sbuf_tensor("part", [C, 2], dt.float32))
        zb = en(nc.sbuf_tensor("zb", [C, 1], dt.float32))
        ps = [en(nc.psum_tensor(f"ps{i}", [C, H], dt.float32)) for i in range(2)]
        psj = en(nc.psum_tensor("psj", [128, 512], dt.float32))

        sem_init = en(nc.semaphore("sem_init"))
        # one semaphore per concurrently-in-flight DMA stream: a wait on a
        # semaphore fed by several unordered DMA completions is unsound
        sem_vta = en(nc.semaphore("sem_vta"))
        sem_vtb = en(nc.semaphore("sem_vtb"))
        sem_b1 = en(nc.semaphore("sem_b1"))
        sem_w2 = en(nc.semaphore("sem_w2"))
        w1sem = [en(nc.semaphore(f"sem_w1s{i}")) for i in range(NSLAB)]
        sem_chunk = en(nc.semaphore("sem_chunk"))
        sem_pe = en(nc.semaphore("sem_pe"))
        sem_relu = en(nc.semaphore("sem_relu"))
        sem_reluA = en(nc.semaphore("sem_reluA"))
        sem_mul = en(nc.semaphore("sem_mul"))
        sem_l2 = en(nc.semaphore("sem_l2"))
        sem_out = en(nc.semaphore("sem_out"))

        with nc.Block(no_gpsimd_drain=True) as block:

            @block.gpsimd
            def _(gpsimd):
                # W1 slab stream: issue-and-forget, consumption order
                for i, g, k0, kc in chunks:
                    gpsimd.dma_start(
                        w1[:, g, k0:k0 + kc, :],
                        w1d[g, :, k0:k0 + kc, :],
                    ).then_inc(w1sem[i], 16)

            @block.sync
            def _(sync):
                # sync's program starts earliest (~6.2us).  vt is split so
                # the PE's first k-tiles don't wait for the whole batch;
                # the ones row (k index KT) is memset on-device instead of
                # transferred, so the bias matmul only needs b1.
                sync.dma_start(b1[:], b1d[:]).then_inc(sem_b1, 16)
                sync.dma_start(vt[:, 0:3], vtd[:, 0:3]).then_inc(sem_vta, 16)
                sync.dma_start(vt[:, 3:KT], vtd[:, 3:KT]).then_inc(sem_vtb, 16)
                sync.dma_start(w2[:], w2d[:]).then_inc(sem_w2, 16)
                sync.wait_ge(sem_l2, 2 * G)
                sync.dma_start(outd[:], logits[:]).then_inc(sem_out, 16)
                # no final receipt wait: the SP drain at block exit flushes
                # the HWDGE queue before the NEFF retires

            @block.scalar
            def _(scalar):
                scalar.wait_ge(sem_init, 1)  # zb ready
                for g in range(G):
                    scalar.wait_ge(sem_chunk, g + 1)
                    if g >= 2:
                        # o1 slot free once DVE finished expert g-2's ttrs
                        scalar.wait_ge(sem_mul, 4 * g - 4)
                    scalar.activation(
                        o1[:, g % 2, 0:512], ps[g % 2][:, 0:512],
                        Act.Relu, bias=zb[:]).then_inc(sem_reluA, 1)
                    scalar.activation(
                        o1[:, g % 2, 512:H], ps[g % 2][:, 512:H],
                        Act.Relu, bias=zb[:]).then_inc(sem_relu, 1)
                    # layer-2 reduction, t=1 half: in-place Copy with
                    # accum_out sums along the free axis (t=0 runs on DVE);
                    # muls are ordered t1-first so this starts after mul #3
                    scalar.wait_ge(sem_mul, 4 * g + 3)
                    scalar.activation(
                        scr[:, 1, :], scr[:, 1, :], Act.Copy,
                        accum_out=logits[:, 2 * g + 1:2 * g + 2]).then_inc(
                            sem_l2, 1)

            @block.tensor
            def _(tensor):
                # ungated warmup on SBUF garbage: results land in psj and
                # are never read.  The PE clock reaches full speed ~5-6us
                # after the start of CONTINUOUS activity and a gap resets
                # the ramp, so the burst is sized to bridge from engine
                # boot (~6.3us) to vt/b1 arrival with no idle window.
                # full-width (M=128) dummies draw maximal PE power to pull
                # the DVFS boost forward
                for _ in range(N_WARMUP_512):
                    tensor.matmul(psj[:], wz[:, 0:128], wz[:],
                                  start=True, stop=True, skip_group_check=True)
                for _ in range(N_WARMUP_256):
                    tensor.matmul(psj[:, 0:256], wz[:, 0:128], wz[:, 0:256],
                                  start=True, stop=True, skip_group_check=True)
                tensor.wait_ge(sem_init, 2)  # ones row of vt
                tensor.wait_ge(sem_b1, 16)

                def bias(g):
                    # starts the accumulation group so the expert's tail
                    # ends on its last W1 k-tile's matmul
                    for lo, hi in regions:
                        tensor.matmul(
                            ps[g % 2][:, lo:hi], vt[0:1, KT, g, :],
                            b1[0:1, g, lo:hi], start=True, stop=False,
                            skip_group_check=True)

                gch = [[c for c in chunks if c[1] == g] for g in range(G)]
                bias(0)
                for g in range(G):
                    for ci, (i, _, k0, kc) in enumerate(gch[g]):
                        if g == 0 and k0 == 0:
                            tensor.wait_ge(sem_vta, 16)
                        if g == 0 and k0 == 3:
                            tensor.wait_ge(sem_vtb, 16)
                        if ci == len(gch[g]) - 1 and g + 1 < G:
                            # hoist the next expert's bias into this k-
                            # stream, BEFORE the last slab's wait so the
                            # bias matmuls execute during any supply stall:
                            # by now relu of g-1 has long freed the other
                            # PSUM slot (regions alternate per k so
                            # consecutive matmuls hit different PSUM banks)
                            tensor.wait_ge(sem_relu, g)
                            bias(g + 1)
                        tensor.wait_ge(w1sem[i], 16)
                        for kk in range(kc):
                            k = k0 + kk
                            for lo, hi in regions:
                                inst = tensor.matmul(
                                    ps[g % 2][:, lo:hi],
                                    vt[:, k, g, :],
                                    w1[:, g, k, lo:hi],
                                    start=False, stop=(k == KT - 1),
                                )
                    # fires at retirement: scalar's relu waits on this
                    inst.then_inc(sem_chunk, 1)

            @block.vector
            def _(vector):
                vector.memset(zb[:], 0.0).then_inc(sem_init, 1)
                vector.memset(vt[0:1, KT, :, :], 1.0).then_inc(sem_init, 1)
                vector.memset(o1[:, 0, H:H + 1], 1.0)
                vector.memset(o1[:, 1, H:H + 1], 1.0)
                vector.wait_ge(sem_w2, 16)
                for g in range(G):
                    # lo-region muls overlap ACT's hi-region relu
                    vector.wait_ge(sem_reluA, g + 1)
                    if g >= 1:
                        # scr slots free once both g-1 reductions are done
                        vector.wait_ge(sem_l2, 2 * g)
                    for t in (1, 0):
                        vector.tensor_mul(
                            scr[:, t, 0:512], o1[:, g % 2, 0:512],
                            w2[:, g, t, 0:512]).then_inc(sem_mul, 1)
                    vector.wait_ge(sem_relu, g + 1)
                    for t in (1, 0):
                        vector.tensor_mul(
                            scr[:, t, 512:H + 1], o1[:, g % 2, 512:H + 1],
                            w2[:, g, t, 512:H + 1]).then_inc(sem_mul, 1)
                    # t=0 reduction on DVE in parallel with ACT's t=1 half;
                    # the sem_mul wait makes it completion-tied to the muls
                    vector.wait_ge(sem_mul, 4 * g + 4)
                    vector.reduce_sum(
                        logits[:, 2 * g:2 * g + 1], scr[:, 0, :],
                        axis=mybir.AxisListType.X).then_inc(sem_l2, 1)

    # Strip the ctor-emitted all-engine barrier (incl. a ~3us GpSimd
    # dge-drain) from `main`: nothing reads the const APs it fences, and
    # every cross-engine dependency in this kernel is explicitly
    # semaphored, so the input streams can start as soon as engines boot.
    main_bb = nc.m.functions[0].blocks[0]

    def _is_ctor_barrier(inst):
        if type(inst).__name__ == "InstDrain":
            return True
        si = inst.sync_info
        if si is None:
            return False
        names = [u.ant_name or "" for u in (si.on_update or [])]
        names += [getattr(w, "ant_name", "") or "" for w in (si.on_wait or [])]
        return any(n.startswith("barrier_") for n in names)

    kept = [i for i in main_bb.instructions if not _is_ctor_barrier(i)]
    if len(kept) != len(main_bb.instructions):
        main_bb.instructions[:] = kept

    nc.compile()
    return nc


def _route(ids: np.ndarray, n_experts: int):
    """Group sample indices by expert; split groups >64; pad count to 8k."""
    CAP = 64
    groups = []
    for e in range(n_experts):
        idx = np.nonzero(ids == e)[0]
        if len(idx) <= CAP:
            groups.append((e, idx))
        else:
            for j in range(0, len(idx), CAP):
                groups.append((e, idx[j:j + CAP]))
    while len(groups) % N_CORES:
        groups.append((0, np.empty(0, np.int64)))
    G = len(groups) // N_CORES
    C = max(max((len(i) for _, i in groups)), 1)
    return groups, G, C


def _run(inputs: dict, trace: bool = False, **run_kwargs):
    v_X = np.asarray(inputs["v_X"], dtype=np.float32)
    v_H = np.asarray(inputs["v_H"], dtype=np.float32)
    ids = np.asarray(inputs["aspect_ids"]).astype(np.int64)
    W1 = np.asarray(inputs["W1_embs"], dtype=np.float32)
    b1 = np.asarray(inputs["b1_embs"], dtype=np.float32)
    W2 = np.asarray(inputs["W2_embs"], dtype=np.float32)
    b2 = np.asarray(inputs["b2_embs"], dtype=np.float32)

    B = v_X.shape[0]
    A = W1.shape[0]
    V = np.concatenate([v_X, v_H], axis=1)  # (B, D)
    assert V.shape[1] == D and b1.shape[1] == H

    groups, G, C = _route(ids, A)

    key = (G, C)
    if key not in _graph_cache:
        _graph_cache[key] = _build(G, C)
    nc = _graph_cache[key]

    bf16 = ml_dtypes.bfloat16
    f8 = ml_dtypes.float8_e3m4
    sw = np.float32(W1_SCALE)
    in_maps = []
    for c in range(N_CORES):
        cg = groups[c * G:(c + 1) * G]
        # [g, p, k, h] layout, x128 into e3m4 (amax*128 = 13.9 < 15.5 max)
        w1c = np.stack([(W1[e] * sw).reshape(KT, 128, H).transpose(1, 0, 2)
                        for e, _ in cg]).astype(f8)
        vtc = np.zeros((128, KT + 1, G, C), dtype=bf16)
        w2c = np.zeros((C, G, 2, H + 1), dtype=bf16)
        b1c = (np.stack([b1[e] for e, _ in cg])[None] * sw).astype(bf16)
        for g, (e, idx) in enumerate(cg):
            n = len(idx)
            if n:
                # V[idx].T: (D, n) -> (KT, 128, n) -> [p, k, c]
                vtc[:, :KT, g, :n] = (
                    V[idx].T.reshape(KT, 128, n).transpose(1, 0, 2).astype(bf16))
            w2r = W2[e].reshape(H, 2) / sw  # undo the W1 scale after relu
            w2c[:, g, 0, :H] = w2r[:, 0].astype(bf16)
            w2c[:, g, 1, :H] = w2r[:, 1].astype(bf16)
            w2c[:, g, 0, H] = b2[e, 0]
            w2c[:, g, 1, H] = b2[e, 1]
        in_maps.append({
            "w1": np.ascontiguousarray(w1c),
            "vt": np.ascontiguousarray(vtc),
            "b1r": np.ascontiguousarray(b1c),
            "w2e": np.ascontiguousarray(w2c),
        })

    res = run_bass_kernel_spmd(nc, in_maps, core_ids=list(range(N_CORES)),
                               trace=trace, **run_kwargs)

    logits = np.zeros((B, 2), dtype=np.float32)
    for c in range(N_CORES):
        out_c = res.results[c]["out"]  # (C, 2G)
        for g, (e, idx) in enumerate(groups[c * G:(c + 1) * G]):
            n = len(idx)
            if n:
                logits[idx] = out_c[:n, 2 * g:2 * g + 2]
    return logits, res


def kernel(**inputs) -> np.ndarray:
    logits, _ = _run(inputs, trace=False)
    return logits


# revision 50
# speedup vs baseline: 1.0301x; 1.0301x over previous
"""Trainium2 Bass kernel for per-sample expert-routed 2-layer MLP (MoE routing).

Problem: logits[b] = relu(V[b] @ W1[id[b]] + b1[id[b]]) @ W2[id[b]] + b2[id[b]]
  V = concat(v_X, v_H): (256, 1536), 32 experts, W1 per expert (1536, 768).

Strategy (expert parallel over 8 NeuronCores, raw bacc pipeline):
  - Host routes samples to experts, assigns 4 experts per core, and casts
    W1 to float8_e3m4 (x128 scale, folded out of b1/W2 on host), so each
    expert's 1.18 MB streams HBM->SBUF exactly once chip-wide at one byte
    per element -- half the bf16 traffic.  The whole per-core W1 (4.7 MB
    = 36.9 KB/partition) is SBUF-resident: no ring, no recycle waits.
  - W1 slab DMAs issue from the sync engine (fine-grained for expert 0 so
    the PE starts early, 6-k-tile slabs after -- dma_start issue costs
    ~750ns + 5ns/descriptor of sequencer time, so big slabs keep the
    stream issue-rate above the queue bandwidth).  vt/b1/w2 issue from
    GpSimd, whose Q7 boots earliest.
  - The TensorEngine chases the stream with V^T stationary (bf16; matmul
    operands mix dtypes in normal mode).  PSUM regions 0:512/512:768
    alternate per k-tile: consecutive matmuls into the same PSUM bank
    stall ~330ns, alternating banks run back-to-back.  A dummy-matmul
    burst at boot starts the PE clock ramp before real data lands.
  - Bias starts each PSUM accumulation group as a K=1 matmul against an
    all-ones row.
  - Layer 2 (768 -> 2): four fused multiply-reduce passes on the DVE
    (tensor_tensor_reduce, region-chained via the init-value AP; b2 via a
    ones column), overlapped with the scalar engine's two-region relu.
  - Outputs (capacity-padded per-expert logits) are scattered on host.
"""

from contextlib import ExitStack

import ml_dtypes
import numpy as np

import concourse.bacc as bacc
import concourse.mybir as mybir
from concourse.bass_utils import run_bass_kernel_spmd

N_CORES = 8
KT = 12          # K tiles of 128 over D=1536
D = 1536
H = 768
W1_SCALE = 128.0
N_WARMUP_512 = 9
N_WARMUP_256 = 1
# k-tiles per W1 slab DMA, per expert-group.  Every slab costs 128
# descriptors (~0.1-0.25us of queue processing each, plus ~0.8us of
# sequencer issue), so k-tile SUPPLY RATE scales with slab size: the
# head matches the mid-clock PE rate, late experts ship whole to cut
# total descriptor count (and so total queue time).
K_CHUNKS = ((1, 2, 3, 3, 3), (6, 6), (6, 6), (6, 6))

_graph_cache = {}


def _build(G: int, C: int):
    """Build the SPMD graph: G expert-groups per core, capacity C samples."""
    dt = mybir.dt
    Act = mybir.ActivationFunctionType
    Alu = mybir.AluOpType

    nc = bacc.Bacc("TRN2", target_bir_lowering=False, debug=False,
                   enable_asserts=False, monotonic_sem_count=0)

    chunks = []  # (global_id, g, k0, kc)
    for g in range(G):
        for kc in K_CHUNKS[g] if g < len(K_CHUNKS) else K_CHUNKS[-1]:
            k0 = sum(c[3] for c in chunks if c[1] == g)
            chunks.append((len(chunks), g, k0, kc))
    NSLAB = len(chunks)

    w1d = nc.dram_tensor("w1", [G, 128, KT, H], dt.float8e3, kind="ExternalInput")
    vtd = nc.dram_tensor("vt", [128, KT + 1, G, C], dt.bfloat16, kind="ExternalInput")
    b1d = nc.dram_tensor("b1r", [1, G, H], dt.bfloat16, kind="ExternalInput")
    b1bd = nc.dram_tensor("b1b", [C, G, H], dt.bfloat16, kind="ExternalInput")
    w2d = nc.dram_tensor("w2e", [C, G, 2, H + 1], dt.bfloat16, kind="ExternalInput")
    outd = nc.dram_tensor("out", [C, 2 * G], dt.float32, kind="ExternalOutput")

    regions = ((0, 512), (512, H))

    with ExitStack() as ctx:
        en = ctx.enter_context
        wz = en(nc.sbuf_tensor("wz", [128, 512], dt.bfloat16))
        w1 = en(nc.sbuf_tensor("w1_sb", [128, G, KT, H], dt.float8e3))
        vt = en(nc.sbuf_tensor("vt_sb", [128, KT + 1, G, C], dt.bfloat16))
        b1 = en(nc.sbuf_tensor("b1_sb", [1, G, H], dt.bfloat16))
        b1b = en(nc.sbuf_tensor("b1b_sb", [C, G, H], dt.bfloat16))
        w2 = en(nc.sbuf_tensor("w2_sb", [C, G, 2, H + 1], dt.bfloat16))
        o1 = en(nc.sbuf_tensor("o1", [C, 2, H + 1], dt.bfloat16))
        scr = en(nc.sbuf_tensor("scr", [C, 2, H + 1], dt.bfloat16))
        logits = en(nc.sbuf_tensor("logits", [C, 2 * G], dt.float32))
        part = en(nc.sbuf_tensor("part", [C, 2], dt.float32))
        zb = en(nc.sbuf_tensor("zb", [C, 1], dt.float32))
        ps = [en(nc.psum_tensor(f"ps{i}", [C, H], dt.float32)) for i in range(2)]
        psj = en(nc.psum_tensor("psj", [128, 512], dt.float32))

        sem_init = en(nc.semaphore("sem_init"))
        # one semaphore per concurrently-in-flight DMA stream: a wait on a
        # semaphore fed by several unordered DMA completions is unsound
        sem_vta = en(nc.semaphore("sem_vta"))
        sem_vtb = en(nc.semaphore("sem_vtb"))
        sem_b1 = en(nc.semaphore("sem_b1"))
        sem_w2 = en(nc.semaphore("sem_w2"))
        w1sem = [en(nc.semaphore(f"sem_w1s{i}")) for i in range(NSLAB)]
        sem_chunk = en(nc.semaphore("sem_chunk"))
        sem_pe = en(nc.semaphore("sem_pe"))
        sem_relu = en(nc.semaphore("sem_relu"))
        sem_reluA = en(nc.semaphore("sem_reluA"))
        sem_mul = en(nc.semaphore("sem_mul"))
        sem_l2 = en(nc.semaphore("sem_l2"))
        sem_out = en(nc.semaphore("sem_out"))
        sem_b1b = en(nc.semaphore("sem_b1b"))
        sem_bw = en(nc.semaphore("sem_bw"))

        with nc.Block(no_gpsimd_drain=True) as block:

            @block.gpsimd
            def _(gpsimd):
                # W1 slab stream: issue-and-forget, consumption order
                for i, g, k0, kc in chunks:
                    gpsimd.dma_start(
                        w1[:, g, k0:k0 + kc, :],
                        w1d[g, :, k0:k0 + kc, :],
                    ).then_inc(w1sem[i], 16)

            @block.sync
            def _(sync):
                # sync's program starts earliest (~6.2us).  vt is split so
                # the PE's first k-tiles don't wait for the whole batch;
                # the ones row (k index KT) is memset on-device instead of
                # transferred, so the bias matmul only needs b1.
                sync.dma_start(b1[:], b1d[:]).then_inc(sem_b1, 16)
                sync.dma_start(vt[:, 0:3], vtd[:, 0:3]).then_inc(sem_vta, 16)
                sync.dma_start(b1b[:], b1bd[:]).then_inc(sem_b1b, 16)
                sync.dma_start(vt[:, 3:KT], vtd[:, 3:KT]).then_inc(sem_vtb, 16)
                sync.dma_start(w2[:], w2d[:]).then_inc(sem_w2, 16)
                sync.wait_ge(sem_l2, 2 * G)
                sync.dma_start(outd[:], logits[:]).then_inc(sem_out, 16)
                # no final receipt wait: the SP drain at block exit flushes
                # the HWDGE queue before the NEFF retires

            @block.scalar
            def _(scalar):
                # experts 1..G-1 get their bias seeded into PSUM by this
                # engine (plain Copy of the host-broadcast b1), so the PE's
                # k-matmuls accumulate on top with start=False -- taking
                # the bias matmuls off the TensorEngine's stream
                scalar.wait_ge(sem_init, 1)  # zb ready
                scalar.wait_ge(sem_b1b, 16)
                for lo, hi in regions:
                    inst = scalar.activation(
                        ps[1][:, lo:hi], b1b[:, 1, lo:hi], Act.Copy)
                inst.then_inc(sem_bw, 1)
                for g in range(G):
                    scalar.wait_ge(sem_chunk, g + 1)
                    if g >= 2:
                        # o1 slot free once DVE finished expert g-2's ttrs
                        scalar.wait_ge(sem_mul, 4 * g - 4)
                    scalar.activation(
                        o1[:, g % 2, 0:512], ps[g % 2][:, 0:512],
                        Act.Relu, bias=zb[:]).then_inc(sem_reluA, 1)
                    scalar.activation(
                        o1[:, g % 2, 512:H], ps[g % 2][:, 512:H],
                        Act.Relu, bias=zb[:]).then_inc(sem_relu, 1)
                    if g + 2 < G:
                        # ps[g%2] was just read by the relu above; reseed it
                        # with expert g+2's bias (program order makes this
                        # safe on this engine)
                        for lo, hi in regions:
                            inst = scalar.activation(
                                ps[g % 2][:, lo:hi], b1b[:, g + 2, lo:hi],
                                Act.Copy)
                        inst.then_inc(sem_bw, 1)
                    # layer-2 reduction, t=1 half: in-place Copy with
                    # accum_out sums along the free axis (t=0 runs on DVE);
                    # muls are ordered t1-first so this starts after mul #3
                    scalar.wait_ge(sem_mul, 4 * g + 3)
                    scalar.activation(
                        scr[:, 1, :], scr[:, 1, :], Act.Copy,
                        accum_out=logits[:, 2 * g + 1:2 * g + 2]).then_inc(
                            sem_l2, 1)

            @block.tensor
            def _(tensor):
                # ungated warmup on SBUF garbage: results land in psj and
                # are never read.  The PE clock reaches full speed ~5-6us
                # after the start of CONTINUOUS activity and a gap resets
                # the ramp, so the burst is sized to bridge from engine
                # boot (~6.3us) to vt/b1 arrival with no idle window.
                # full-width (M=128) dummies draw maximal PE power to pull
                # the DVFS boost forward
                for _ in range(N_WARMUP_512):
                    tensor.matmul(psj[:], wz[:, 0:128], wz[:],
                                  start=True, stop=True, skip_group_check=True)
                for _ in range(N_WARMUP_256):
                    tensor.matmul(psj[:, 0:256], wz[:, 0:128], wz[:, 0:256],
                                  start=True, stop=True, skip_group_check=True)
                tensor.wait_ge(sem_init, 2)  # ones row of vt
                tensor.wait_ge(sem_b1, 16)

                def bias(g):
                    # starts the accumulation group so the expert's tail
                    # ends on its last W1 k-tile's matmul
                    for lo, hi in regions:
                        tensor.matmul(
                            ps[g % 2][:, lo:hi], vt[0:1, KT, g, :],
                            b1[0:1, g, lo:hi], start=True, stop=False,
                            skip_group_check=True)

                gch = [[c for c in chunks if c[1] == g] for g in range(G)]
                bias(0)
                for g in range(G):
                    if g >= 1:
                        # bias for this expert was seeded into PSUM by the
                        # scalar engine
                        tensor.wait_ge(sem_bw, g)
                    for ci, (i, _, k0, kc) in enumerate(gch[g]):
                        if g == 0 and k0 == 0:
                            tensor.wait_ge(sem_vta, 16)
                        if g == 0 and k0 == 3:
                            tensor.wait_ge(sem_vtb, 16)
                        tensor.wait_ge(w1sem[i], 16)
                        for kk in range(kc):
                            k = k0 + kk
                            # regions alternate per k so consecutive
                            # matmuls hit different PSUM banks
                            for lo, hi in regions:
                                inst = tensor.matmul(
                                    ps[g % 2][:, lo:hi],
                                    vt[:, k, g, :],
                                    w1[:, g, k, lo:hi],
                                    start=False, stop=(k == KT - 1),
                                    skip_group_check=(g >= 1),
                                )
                    # fires at retirement: scalar's relu waits on this
                    inst.then_inc(sem_chunk, 1)

            @block.vector
            def _(vector):
                vector.memset(zb[:], 0.0).then_inc(sem_init, 1)
                vector.memset(vt[0:1, KT, :, :], 1.0).then_inc(sem_init, 1)
                vector.memset(o1[:, 0, H:H + 1], 1.0)
                vector.memset(o1[:, 1, H:H + 1], 1.0)
                vector.wait_ge(sem_w2, 16)
                for g in range(G):
                    # lo-region muls overlap ACT's hi-region relu
                    vector.wait_ge(sem_reluA, g + 1)
                    if g >= 1:
                        # scr slots free once both g-1 reductions are done
                        vector.wait_ge(sem_l2, 2 * g)
                    for t in (1, 0):
                        vector.tensor_mul(
                            scr[:, t, 0:512], o1[:, g % 2, 0:512],
                            w2[:, g, t, 0:512]).then_inc(sem_mul, 1)
                    vector.wait_ge(sem_relu, g + 1)
                    for t in (1, 0):
                        vector.tensor_mul(
                            scr[:, t, 512:H + 1], o1[:, g % 2, 512:H + 1],
                            w2[:, g, t, 512:H + 1]).then_inc(sem_mul, 1)
                    # t=0 reduction on DVE in parallel with ACT's t=1 half;
                    # the sem_mul wait makes it completion-tied to the muls
                    vector.wait_ge(sem_mul, 4 * g + 4)
                    vector.reduce_sum(
                        logits[:, 2 * g:2 * g + 1], scr[:, 0, :],
                        axis=mybir.AxisListType.X).then_inc(sem_l2, 1)

    # Strip the ctor-emitted all-engine barrier (incl. a ~3us GpSimd
    # dge-drain) from `main`: nothing reads the const APs it fences, and
    # every cross-engine dependency in this kernel is explicitly
    # semaphored, so the input streams can start as soon as engines boot.
    main_bb = nc.m.functions[0].blocks[0]

    def _is_ctor_barrier(inst):
        if type(inst).__name__ == "InstDrain":
            return True
        si = inst.sync_info
        if si is None:
            return False
        names = [u.ant_name or "" for u in (si.on_update or [])]
        names += [getattr(w, "ant_name", "") or "" for w in (si.on_wait or [])]
        return any(n.startswith("barrier_") for n in names)

    kept = [i for i in main_bb.instructions if not _is_ctor_barrier(i)]
    if len(kept) != len(main_bb.instructions):
        main_bb.instructions[:] = kept

    nc.compile()
    return nc


def _route(ids: np.ndarray, n_experts: int):
    """Group sample indices by expert; split groups >64; pad count to 8k."""
    CAP = 64
    groups = []
    for e in range(n_experts):
        idx = np.nonzero(ids == e)[0]
        if len(idx) <= CAP:
            groups.append((e, idx))
        else:
            for j in range(0, len(idx), CAP):
                groups.append((e, idx[j:j + CAP]))
    while len(groups) % N_CORES:
        groups.append((0, np.empty(0, np.int64)))
    G = len(groups) // N_CORES
    C = max(max((len(i) for _, i in groups)), 1)
    return groups, G, C


def _run(inputs: dict, trace: bool = False, **run_kwargs):
    v_X = np.asarray(inputs["v_X"], dtype=np.float32)
    v_H = np.asarray(inputs["v_H"], dtype=np.float32)
    ids = np.asarray(inputs["aspect_ids"]).astype(np.int64)
    W1 = np.asarray(inputs["W1_embs"], dtype=np.float32)
    b1 = np.asarray(inputs["b1_embs"], dtype=np.float32)
    W2 = np.asarray(inputs["W2_embs"], dtype=np.float32)
    b2 = np.asarray(inputs["b2_embs"], dtype=np.float32)

    B = v_X.shape[0]
    A = W1.shape[0]
    V = np.concatenate([v_X, v_H], axis=1)  # (B, D)
    assert V.shape[1] == D and b1.shape[1] == H

    groups, G, C = _route(ids, A)

    key = (G, C)
    if key not in _graph_cache:
        _graph_cache[key] = _build(G, C)
    nc = _graph_cache[key]

    bf16 = ml_dtypes.bfloat16
    f8 = ml_dtypes.float8_e3m4
    sw = np.float32(W1_SCALE)
    in_maps = []
    for c in range(N_CORES):
        cg = groups[c * G:(c + 1) * G]
        # [g, p, k, h] layout, x128 into e3m4 (amax*128 = 13.9 < 15.5 max)
        w1c = np.stack([(W1[e] * sw).reshape(KT, 128, H).transpose(1, 0, 2)
                        for e, _ in cg]).astype(f8)
        vtc = np.zeros((128, KT + 1, G, C), dtype=bf16)
        w2c = np.zeros((C, G, 2, H + 1), dtype=bf16)
        b1c = (np.stack([b1[e] for e, _ in cg])[None] * sw).astype(bf16)
        b1bc = np.broadcast_to(
            (np.stack([b1[e] for e, _ in cg], axis=0) * sw).astype(bf16),
            (C, G, H)).copy()
        for g, (e, idx) in enumerate(cg):
            n = len(idx)
            if n:
                # V[idx].T: (D, n) -> (KT, 128, n) -> [p, k, c]
                vtc[:, :KT, g, :n] = (
                    V[idx].T.reshape(KT, 128, n).transpose(1, 0, 2).astype(bf16))
            w2r = W2[e].reshape(H, 2) / sw  # undo the W1 scale after relu
            w2c[:, g, 0, :H] = w2r[:, 0].astype(bf16)
            w2c[:, g, 1, :H] = w2r[:, 1].astype(bf16)
            w2c[:, g, 0, H] = b2[e, 0]
            w2c[:, g, 1, H] = b2[e, 1]
        in_maps.append({
            "w1": np.ascontiguousarray(w1c),
            "vt": np.ascontiguousarray(vtc),
            "b1r": np.ascontiguousarray(b1c),
            "b1b": np.ascontiguousarray(b1bc),
            "w2e": np.ascontiguousarray(w2c),
        })

    res = run_bass_kernel_spmd(nc, in_maps, core_ids=list(range(N_CORES)),
                               trace=trace, **run_kwargs)

    logits = np.zeros((B, 2), dtype=np.float32)
    for c in range(N_CORES):
        out_c = res.results[c]["out"]  # (C, 2G)
        for g, (e, idx) in enumerate(groups[c * G:(c + 1) * G]):
            n = len(idx)
            if n:
                logits[idx] = out_c[:n, 2 * g:2 * g + 2]
    return logits, res


def kernel(**inputs) -> np.ndarray:
    logits, _ = _run(inputs, trace=False)
    return logits


# revision 51
# speedup vs baseline: 1.0655x; 1.0343x over previous
"""Trainium2 Bass kernel for per-sample expert-routed 2-layer MLP (MoE routing).

Problem: logits[b] = relu(V[b] @ W1[id[b]] + b1[id[b]]) @ W2[id[b]] + b2[id[b]]
  V = concat(v_X, v_H): (256, 1536), 32 experts, W1 per expert (1536, 768).

Strategy (expert parallel over 8 NeuronCores, raw bacc pipeline):
  - Host routes samples to experts, assigns 4 experts per core, and casts
    W1 to float8_e3m4 (x128 scale, folded out of b1/W2 on host), so each
    expert's 1.18 MB streams HBM->SBUF exactly once chip-wide at one byte
    per element -- half the bf16 traffic.  The whole per-core W1 (4.7 MB
    = 36.9 KB/partition) is SBUF-resident: no ring, no recycle waits.
  - W1 slab DMAs issue from the sync engine (fine-grained for expert 0 so
    the PE starts early, 6-k-tile slabs after -- dma_start issue costs
    ~750ns + 5ns/descriptor of sequencer time, so big slabs keep the
    stream issue-rate above the queue bandwidth).  vt/b1/w2 issue from
    GpSimd, whose Q7 boots earliest.
  - The TensorEngine chases the stream with V^T stationary (bf16; matmul
    operands mix dtypes in normal mode).  PSUM regions 0:512/512:768
    alternate per k-tile: consecutive matmuls into the same PSUM bank
    stall ~330ns, alternating banks run back-to-back.  A dummy-matmul
    burst at boot starts the PE clock ramp before real data lands.
  - Bias starts each PSUM accumulation group as a K=1 matmul against an
    all-ones row.
  - Layer 2 (768 -> 2): four fused multiply-reduce passes on the DVE
    (tensor_tensor_reduce, region-chained via the init-value AP; b2 via a
    ones column), overlapped with the scalar engine's two-region relu.
  - Outputs (capacity-padded per-expert logits) are scattered on host.
"""

from contextlib import ExitStack

import ml_dtypes
import numpy as np

import concourse.bacc as bacc
import concourse.mybir as mybir
from concourse.bass_utils import run_bass_kernel_spmd

N_CORES = 8
KT = 12          # K tiles of 128 over D=1536
D = 1536
H = 768
W1_SCALE = 128.0
N_WARMUP_512 = 5
N_WARMUP_256 = 1
# k-tiles per W1 slab DMA, per expert-group.  Every slab costs 128
# descriptors (~0.1-0.25us of queue processing each, plus ~0.8us of
# sequencer issue), so k-tile SUPPLY RATE scales with slab size: the
# head matches the mid-clock PE rate, late experts ship whole to cut
# total descriptor count (and so total queue time).
K_CHUNKS = ((1, 2, 3, 3, 3), (6, 6), (6, 6), (6, 6))

_graph_cache = {}


def _build(G: int, C: int):
    """Build the SPMD graph: G expert-groups per core, capacity C samples."""
    dt = mybir.dt
    Act = mybir.ActivationFunctionType
    Alu = mybir.AluOpType

    nc = bacc.Bacc("TRN2", target_bir_lowering=False, debug=False,
                   enable_asserts=False, monotonic_sem_count=0)

    chunks = []  # (global_id, g, k0, kc)
    for g in range(G):
        for kc in K_CHUNKS[g] if g < len(K_CHUNKS) else K_CHUNKS[-1]:
            k0 = sum(c[3] for c in chunks if c[1] == g)
            chunks.append((len(chunks), g, k0, kc))
    NSLAB = len(chunks)

    w1d = nc.dram_tensor("w1", [G, 128, KT, H], dt.float8e3, kind="ExternalInput")
    vtd = nc.dram_tensor("vt", [128, KT + 1, G, C], dt.bfloat16, kind="ExternalInput")
    b1d = nc.dram_tensor("b1r", [1, G, H], dt.bfloat16, kind="ExternalInput")
    b1bd = nc.dram_tensor("b1b", [C, G, H], dt.bfloat16, kind="ExternalInput")
    w2d = nc.dram_tensor("w2e", [C, G, 2, H + 1], dt.bfloat16, kind="ExternalInput")
    outd = nc.dram_tensor("out", [C, 2 * G], dt.float32, kind="ExternalOutput")

    regions = ((0, 512), (512, H))

    with ExitStack() as ctx:
        en = ctx.enter_context
        wz = en(nc.sbuf_tensor("wz", [128, 512], dt.bfloat16))
        w1 = en(nc.sbuf_tensor("w1_sb", [128, G, KT, H], dt.float8e3))
        vt = en(nc.sbuf_tensor("vt_sb", [128, KT + 1, G, C], dt.bfloat16))
        b1 = en(nc.sbuf_tensor("b1_sb", [1, G, H], dt.bfloat16))
        b1b = en(nc.sbuf_tensor("b1b_sb", [C, G, H], dt.bfloat16))
        w2 = en(nc.sbuf_tensor("w2_sb", [C, G, 2, H + 1], dt.bfloat16))
        o1 = en(nc.sbuf_tensor("o1", [C, 2, H + 1], dt.bfloat16))
        scr = en(nc.sbuf_tensor("scr", [C, 2, H + 1], dt.bfloat16))
        logits = en(nc.sbuf_tensor("logits", [C, 2 * G], dt.float32))
        part = en(nc.sbuf_tensor("part", [C, 2], dt.float32))
        zb = en(nc.sbuf_tensor("zb", [C, 1], dt.float32))
        ps = [en(nc.psum_tensor(f"ps{i}", [C, H], dt.float32)) for i in range(2)]
        psj = en(nc.psum_tensor("psj", [128, 512], dt.float32))

        sem_init = en(nc.semaphore("sem_init"))
        # one semaphore per concurrently-in-flight DMA stream: a wait on a
        # semaphore fed by several unordered DMA completions is unsound
        sem_vta = en(nc.semaphore("sem_vta"))
        sem_vtb = en(nc.semaphore("sem_vtb"))
        sem_b1 = en(nc.semaphore("sem_b1"))
        sem_w2 = en(nc.semaphore("sem_w2"))
        w1sem = [en(nc.semaphore(f"sem_w1s{i}")) for i in range(NSLAB)]
        sem_chunk = en(nc.semaphore("sem_chunk"))
        sem_pe = en(nc.semaphore("sem_pe"))
        sem_relu = en(nc.semaphore("sem_relu"))
        sem_reluA = en(nc.semaphore("sem_reluA"))
        sem_mul = en(nc.semaphore("sem_mul"))
        sem_l2 = en(nc.semaphore("sem_l2"))
        sem_out = en(nc.semaphore("sem_out"))
        sem_b1b = en(nc.semaphore("sem_b1b"))
        sem_bw = en(nc.semaphore("sem_bw"))

        with nc.Block(no_gpsimd_drain=True) as block:

            @block.gpsimd
            def _(gpsimd):
                # W1 slab stream: issue-and-forget, consumption order
                for i, g, k0, kc in chunks:
                    gpsimd.dma_start(
                        w1[:, g, k0:k0 + kc, :],
                        w1d[g, :, k0:k0 + kc, :],
                    ).then_inc(w1sem[i], 16)

            @block.sync
            def _(sync):
                # sync's program starts earliest (~6.2us).  vt is split so
                # the PE's first k-tiles don't wait for the whole batch;
                # the ones row (k index KT) is memset on-device instead of
                # transferred, so the bias matmul only needs b1.
                sync.dma_start(b1[:], b1d[:]).then_inc(sem_b1, 16)
                sync.dma_start(vt[:, 0:3], vtd[:, 0:3]).then_inc(sem_vta, 16)
                sync.dma_start(b1b[:], b1bd[:]).then_inc(sem_b1b, 16)
                sync.dma_start(vt[:, 3:KT], vtd[:, 3:KT]).then_inc(sem_vtb, 16)
                sync.dma_start(w2[:], w2d[:]).then_inc(sem_w2, 16)
                sync.wait_ge(sem_l2, 2 * G)
                sync.dma_start(outd[:], logits[:]).then_inc(sem_out, 16)
                # no final receipt wait: the SP drain at block exit flushes
                # the HWDGE queue before the NEFF retires

            @block.scalar
            def _(scalar):
                # experts 1..G-1 get their bias seeded into PSUM by this
                # engine (plain Copy of the host-broadcast b1), so the PE's
                # k-matmuls accumulate on top with start=False -- taking
                # the bias matmuls off the TensorEngine's stream
                scalar.wait_ge(sem_init, 1)  # zb ready
                scalar.wait_ge(sem_b1b, 16)
                for lo, hi in regions:
                    inst = scalar.activation(
                        ps[1][:, lo:hi], b1b[:, 1, lo:hi], Act.Copy)
                inst.then_inc(sem_bw, 1)
                for g in range(G):
                    scalar.wait_ge(sem_chunk, g + 1)
                    if g >= 2:
                        # o1 slot free once DVE finished expert g-2's ttrs
                        scalar.wait_ge(sem_mul, 4 * g - 4)
                    scalar.activation(
                        o1[:, g % 2, 0:512], ps[g % 2][:, 0:512],
                        Act.Relu, bias=zb[:]).then_inc(sem_reluA, 1)
                    scalar.activation(
                        o1[:, g % 2, 512:H], ps[g % 2][:, 512:H],
                        Act.Relu, bias=zb[:]).then_inc(sem_relu, 1)
                    if g + 2 < G:
                        # ps[g%2] was just read by the relu above; reseed it
                        # with expert g+2's bias (program order makes this
                        # safe on this engine)
                        for lo, hi in regions:
                            inst = scalar.activation(
                                ps[g % 2][:, lo:hi], b1b[:, g + 2, lo:hi],
                                Act.Copy)
                        inst.then_inc(sem_bw, 1)
                    # layer-2 reduction, t=1 half: in-place Copy with
                    # accum_out sums along the free axis (t=0 runs on DVE);
                    # muls are ordered t1-first so this starts after mul #3
                    scalar.wait_ge(sem_mul, 4 * g + 3)
                    scalar.activation(
                        scr[:, 1, :], scr[:, 1, :], Act.Copy,
                        accum_out=logits[:, 2 * g + 1:2 * g + 2]).then_inc(
                            sem_l2, 1)

            @block.tensor
            def _(tensor):
                # ungated warmup on SBUF garbage: results land in psj and
                # are never read.  The PE clock reaches full speed ~5-6us
                # after the start of CONTINUOUS activity and a gap resets
                # the ramp, so the burst is sized to bridge from engine
                # boot (~6.3us) to vt/b1 arrival with no idle window.
                # full-width (M=128) dummies draw maximal PE power to pull
                # the DVFS boost forward
                for _ in range(N_WARMUP_512):
                    tensor.matmul(psj[:], wz[:, 0:128], wz[:],
                                  start=True, stop=True, skip_group_check=True)
                for _ in range(N_WARMUP_256):
                    tensor.matmul(psj[:, 0:256], wz[:, 0:128], wz[:, 0:256],
                                  start=True, stop=True, skip_group_check=True)
                tensor.wait_ge(sem_init, 2)  # ones row of vt
                tensor.wait_ge(sem_b1, 16)

                def bias(g):
                    # starts the accumulation group so the expert's tail
                    # ends on its last W1 k-tile's matmul
                    for lo, hi in regions:
                        tensor.matmul(
                            ps[g % 2][:, lo:hi], vt[0:1, KT, g, :],
                            b1[0:1, g, lo:hi], start=True, stop=False,
                            skip_group_check=True)

                gch = [[c for c in chunks if c[1] == g] for g in range(G)]
                bias(0)
                for g in range(G):
                    if g >= 1:
                        # bias for this expert was seeded into PSUM by the
                        # scalar engine
                        tensor.wait_ge(sem_bw, g)
                    for ci, (i, _, k0, kc) in enumerate(gch[g]):
                        if g == 0 and k0 == 0:
                            tensor.wait_ge(sem_vta, 16)
                        if g == 0 and k0 == 3:
                            tensor.wait_ge(sem_vtb, 16)
                        tensor.wait_ge(w1sem[i], 16)
                        for kk in range(kc):
                            k = k0 + kk
                            # regions alternate per k so consecutive
                            # matmuls hit different PSUM banks
                            for lo, hi in regions:
                                inst = tensor.matmul(
                                    ps[g % 2][:, lo:hi],
                                    vt[:, k, g, :],
                                    w1[:, g, k, lo:hi],
                                    start=False, stop=(k == KT - 1),
                                    skip_group_check=(g >= 1),
                                )
                    # fires at retirement: scalar's relu waits on this
                    inst.then_inc(sem_chunk, 1)

            @block.vector
            def _(vector):
                vector.memset(zb[:], 0.0).then_inc(sem_init, 1)
                vector.memset(vt[0:1, KT, :, :], 1.0).then_inc(sem_init, 1)
                vector.memset(o1[:, 0, H:H + 1], 1.0)
                vector.memset(o1[:, 1, H:H + 1], 1.0)
                vector.wait_ge(sem_w2, 16)
                for g in range(G):
                    # lo-region muls overlap ACT's hi-region relu
                    vector.wait_ge(sem_reluA, g + 1)
                    if g >= 1:
                        # scr slots free once both g-1 reductions are done
                        vector.wait_ge(sem_l2, 2 * g)
                    for t in (1, 0):
                        vector.tensor_mul(
                            scr[:, t, 0:512], o1[:, g % 2, 0:512],
                            w2[:, g, t, 0:512]).then_inc(sem_mul, 1)
                    vector.wait_ge(sem_relu, g + 1)
                    for t in (1, 0):
                        vector.tensor_mul(
                            scr[:, t, 512:H + 1], o1[:, g % 2, 512:H + 1],
                            w2[:, g, t, 512:H + 1]).then_inc(sem_mul, 1)
                    # t=0 reduction on DVE in parallel with ACT's t=1 half;
                    # the sem_mul wait makes it completion-tied to the muls
                    vector.wait_ge(sem_mul, 4 * g + 4)
                    vector.reduce_sum(
                        logits[:, 2 * g:2 * g + 1], scr[:, 0, :],
                        axis=mybir.AxisListType.X).then_inc(sem_l2, 1)

    # Strip the ctor-emitted all-engine barrier (incl. a ~3us GpSimd
    # dge-drain) from `main`: nothing reads the const APs it fences, and
    # every cross-engine dependency in this kernel is explicitly
    # semaphored, so the input streams can start as soon as engines boot.
    main_bb = nc.m.functions[0].blocks[0]

    def _is_ctor_barrier(inst):
        if type(inst).__name__ == "InstDrain":
            return True
        si = inst.sync_info
        if si is None:
            return False
        names = [u.ant_name or "" for u in (si.on_update or [])]
        names += [getattr(w, "ant_name", "") or "" for w in (si.on_wait or [])]
        return any(n.startswith("barrier_") for n in names)

    kept = [i for i in main_bb.instructions if not _is_ctor_barrier(i)]
    if len(kept) != len(main_bb.instructions):
        main_bb.instructions[:] = kept

    nc.compile()
    return nc


def _route(ids: np.ndarray, n_experts: int):
    """Group sample indices by expert; split groups >64; pad count to 8k."""
    CAP = 64
    groups = []
    for e in range(n_experts):
        idx = np.nonzero(ids == e)[0]
        if len(idx) <= CAP:
            groups.append((e, idx))
        else:
            for j in range(0, len(idx), CAP):
                groups.append((e, idx[j:j + CAP]))
    while len(groups) % N_CORES:
        groups.append((0, np.empty(0, np.int64)))
    G = len(groups) // N_CORES
    C = max(max((len(i) for _, i in groups)), 1)
    return groups, G, C


def _run(inputs: dict, trace: bool = False, **run_kwargs):
    v_X = np.asarray(inputs["v_X"], dtype=np.float32)
    v_H = np.asarray(inputs["v_H"], dtype=np.float32)
    ids = np.asarray(inputs["aspect_ids"]).astype(np.int64)
    W1 = np.asarray(inputs["W1_embs"], dtype=np.float32)
    b1 = np.asarray(inputs["b1_embs"], dtype=np.float32)
    W2 = np.asarray(inputs["W2_embs"], dtype=np.float32)
    b2 = np.asarray(inputs["b2_embs"], dtype=np.float32)

    B = v_X.shape[0]
    A = W1.shape[0]
    V = np.concatenate([v_X, v_H], axis=1)  # (B, D)
    assert V.shape[1] == D and b1.shape[1] == H

    groups, G, C = _route(ids, A)

    key = (G, C)
    if key not in _graph_cache:
        _graph_cache[key] = _build(G, C)
    nc = _graph_cache[key]

    bf16 = ml_dtypes.bfloat16
    f8 = ml_dtypes.float8_e3m4
    sw = np.float32(W1_SCALE)
    in_maps = []
    for c in range(N_CORES):
        cg = groups[c * G:(c + 1) * G]
        # [g, p, k, h] layout, x128 into e3m4 (amax*128 = 13.9 < 15.5 max)
        w1c = np.stack([(W1[e] * sw).reshape(KT, 128, H).transpose(1, 0, 2)
                        for e, _ in cg]).astype(f8)
        vtc = np.zeros((128, KT + 1, G, C), dtype=bf16)
        w2c = np.zeros((C, G, 2, H + 1), dtype=bf16)
        b1c = (np.stack([b1[e] for e, _ in cg])[None] * sw).astype(bf16)
        b1bc = np.broadcast_to(
            (np.stack([b1[e] for e, _ in cg], axis=0) * sw).astype(bf16),
            (C, G, H)).copy()
        for g, (e, idx) in enumerate(cg):
            n = len(idx)
            if n:
                # V[idx].T: (D, n) -> (KT, 128, n) -> [p, k, c]
                vtc[:, :KT, g, :n] = (
                    V[idx].T.reshape(KT, 128, n).transpose(1, 0, 2).astype(bf16))
            w2r = W2[e].reshape(H, 2) / sw  # undo the W1 scale after relu
            w2c[:, g, 0, :H] = w2r[:, 0].astype(bf16)
            w2c[:, g, 1, :H] = w2r[:, 1].astype(bf16)
            w2c[:, g, 0, H] = b2[e, 0]
            w2c[:, g, 1, H] = b2[e, 1]
        in_maps.append({
            "w1": np.ascontiguousarray(w1c),
            "vt": np.ascontiguousarray(vtc),
            "b1r": np.ascontiguousarray(b1c),
            "b1b": np.ascontiguousarray(b1bc),
            "w2e": np.ascontiguousarray(w2c),
        })

    res = run_bass_kernel_spmd(nc, in_maps, core_ids=list(range(N_CORES)),
                               trace=trace, **run_kwargs)

    logits = np.zeros((B, 2), dtype=np.float32)
    for c in range(N_CORES):
        out_c = res.results[c]["out"]  # (C, 2G)
        for g, (e, idx) in enumerate(groups[c * G:(c + 1) * G]):
            n = len(idx)
            if n:
                logits[idx] = out_c[:n, 2 * g:2 * g + 2]
    return logits, res


def kernel(**inputs) -> np.ndarray:
    logits, _ = _run(inputs, trace=False)
    return logits


# revision 56
# speedup vs baseline: 1.1414x; 1.0712x over previous
"""Trainium2 Bass kernel for per-sample expert-routed 2-layer MLP (MoE routing).

Problem: logits[b] = relu(V[b] @ W1[id[b]] + b1[id[b]]) @ W2[id[b]] + b2[id[b]]
  V = concat(v_X, v_H): (256, 1536), 32 experts, W1 per expert (1536, 768).

Strategy (expert parallel over 8 NeuronCores, raw bacc pipeline):
  - Host routes samples to experts, assigns 4 experts per core, and casts
    W1 to float8_e3m4 (x128 scale, folded out of b1/W2 on host), so each
    expert's 1.18 MB streams HBM->SBUF exactly once chip-wide at one byte
    per element -- half the bf16 traffic.  The whole per-core W1 (4.7 MB
    = 36.9 KB/partition) is SBUF-resident: no ring, no recycle waits.
  - W1 slab DMAs issue from the sync engine (fine-grained for expert 0 so
    the PE starts early, 6-k-tile slabs after -- dma_start issue costs
    ~750ns + 5ns/descriptor of sequencer time, so big slabs keep the
    stream issue-rate above the queue bandwidth).  vt/b1/w2 issue from
    GpSimd, whose Q7 boots earliest.
  - The TensorEngine chases the stream with V^T stationary (bf16; matmul
    operands mix dtypes in normal mode).  PSUM regions 0:512/512:768
    alternate per k-tile: consecutive matmuls into the same PSUM bank
    stall ~330ns, alternating banks run back-to-back.  A dummy-matmul
    burst at boot starts the PE clock ramp before real data lands.
  - Bias starts each PSUM accumulation group as a K=1 matmul against an
    all-ones row.
  - Layer 2 (768 -> 2): four fused multiply-reduce passes on the DVE
    (tensor_tensor_reduce, region-chained via the init-value AP; b2 via a
    ones column), overlapped with the scalar engine's two-region relu.
  - Outputs (capacity-padded per-expert logits) are scattered on host.
"""

from contextlib import ExitStack

import ml_dtypes
import numpy as np

import concourse.bacc as bacc
import concourse.mybir as mybir
from concourse.bass_utils import run_bass_kernel_spmd

N_CORES = 8
KT = 12          # K tiles of 128 over D=1536
D = 1536
H = 768
W1_SCALE = 128.0
N_WARMUP_512 = 5
N_WARMUP_256 = 1
# k-tiles per W1 slab DMA, per expert-group.  Every slab costs 128
# descriptors (~0.1-0.25us of queue processing each, plus ~0.8us of
# sequencer issue), so k-tile SUPPLY RATE scales with slab size: the
# head matches the mid-clock PE rate, late experts ship whole to cut
# total descriptor count (and so total queue time).
K_CHUNKS = ((1, 2, 3, 3, 3), (6, 6), (6, 6), (6, 6))

_graph_cache = {}


def _build(G: int, C: int):
    """Build the SPMD graph: G expert-groups per core, capacity C samples."""
    dt = mybir.dt
    Act = mybir.ActivationFunctionType
    Alu = mybir.AluOpType

    nc = bacc.Bacc("TRN2", target_bir_lowering=False, debug=False,
                   enable_asserts=False, monotonic_sem_count=0)

    chunks = []  # (global_id, g, k0, kc)
    for g in range(G):
        for kc in K_CHUNKS[g] if g < len(K_CHUNKS) else K_CHUNKS[-1]:
            k0 = sum(c[3] for c in chunks if c[1] == g)
            chunks.append((len(chunks), g, k0, kc))
    NSLAB = len(chunks)

    w1d = nc.dram_tensor("w1", [G, 128, KT, H], dt.float8e3, kind="ExternalInput")
    vtd = nc.dram_tensor("vt", [128, KT + 1, G, C], dt.bfloat16, kind="ExternalInput")
    b1d = nc.dram_tensor("b1r", [1, G, H], dt.bfloat16, kind="ExternalInput")
    w2d = nc.dram_tensor("w2e", [C, G, 2, H + 1], dt.bfloat16, kind="ExternalInput")
    outd = nc.dram_tensor("out", [C, 2 * G], dt.float32, kind="ExternalOutput")

    regions = ((0, 512), (512, H))

    with ExitStack() as ctx:
        en = ctx.enter_context
        wz = en(nc.sbuf_tensor("wz", [128, 512], dt.bfloat16))
        w1 = en(nc.sbuf_tensor("w1_sb", [128, G, KT, H], dt.float8e3))
        vt = en(nc.sbuf_tensor("vt_sb", [128, KT + 1, G, C], dt.bfloat16))
        b1 = en(nc.sbuf_tensor("b1_sb", [1, G, H], dt.bfloat16))
        w2 = en(nc.sbuf_tensor("w2_sb", [C, G, 2, H + 1], dt.bfloat16))
        o1 = en(nc.sbuf_tensor("o1", [C, 2, H + 1], dt.bfloat16))
        scr = en(nc.sbuf_tensor("scr", [C, 2, H + 1], dt.bfloat16))
        logits = en(nc.sbuf_tensor("logits", [C, 2 * G], dt.float32))
        part = en(nc.sbuf_tensor("part", [C, 2], dt.float32))
        zb = en(nc.sbuf_tensor("zb", [C, 1], dt.float32))
        ps = [en(nc.psum_tensor(f"ps{i}", [C, H], dt.float32)) for i in range(2)]
        psj = en(nc.psum_tensor("psj", [128, 512], dt.float32))

        sem_init = en(nc.semaphore("sem_init"))
        # one semaphore per concurrently-in-flight DMA stream: a wait on a
        # semaphore fed by several unordered DMA completions is unsound
        sem_vta = en(nc.semaphore("sem_vta"))
        sem_vtb = en(nc.semaphore("sem_vtb"))
        sem_b1 = en(nc.semaphore("sem_b1"))
        sem_w2 = en(nc.semaphore("sem_w2"))
        w1sem = [en(nc.semaphore(f"sem_w1s{i}")) for i in range(NSLAB)]
        sem_chunk = en(nc.semaphore("sem_chunk"))
        sem_pe = en(nc.semaphore("sem_pe"))
        sem_relu = en(nc.semaphore("sem_relu"))
        sem_reluA = en(nc.semaphore("sem_reluA"))
        sem_mul = en(nc.semaphore("sem_mul"))
        sem_l2 = en(nc.semaphore("sem_l2"))
        sem_out = en(nc.semaphore("sem_out"))

        with nc.Block(no_gpsimd_drain=True) as block:

            @block.gpsimd
            def _(gpsimd):
                # W1 slab stream: issue-and-forget, consumption order
                for i, g, k0, kc in chunks:
                    gpsimd.dma_start(
                        w1[:, g, k0:k0 + kc, :],
                        w1d[g, :, k0:k0 + kc, :],
                    ).then_inc(w1sem[i], 16)

            @block.sync
            def _(sync):
                # sync's program starts earliest (~6.2us).  vt is split so
                # the PE's first k-tiles don't wait for the whole batch;
                # the ones row (k index KT) is memset on-device instead of
                # transferred, so the bias matmul only needs b1.
                sync.dma_start(b1[:], b1d[:]).then_inc(sem_b1, 16)
                sync.dma_start(vt[:, 0:3], vtd[:, 0:3]).then_inc(sem_vta, 16)
                sync.dma_start(vt[:, 3:KT], vtd[:, 3:KT]).then_inc(sem_vtb, 16)
                sync.dma_start(w2[:], w2d[:]).then_inc(sem_w2, 16)
                sync.wait_ge(sem_l2, 2 * G)
                sync.dma_start(outd[:], logits[:]).then_inc(sem_out, 16)
                # no final receipt wait: the SP drain at block exit flushes
                # the HWDGE queue before the NEFF retires

            @block.scalar
            def _(scalar):
                scalar.wait_ge(sem_init, 1)  # zb ready
                for g in range(G):
                    scalar.wait_ge(sem_chunk, g + 1)
                    if g >= 2:
                        # o1 slot free once DVE finished expert g-2's ttrs
                        scalar.wait_ge(sem_mul, 4 * g - 4)
                    scalar.activation(
                        o1[:, g % 2, 0:512], ps[g % 2][:, 0:512],
                        Act.Relu, bias=zb[:]).then_inc(sem_reluA, 1)
                    scalar.activation(
                        o1[:, g % 2, 512:H], ps[g % 2][:, 512:H],
                        Act.Relu, bias=zb[:]).then_inc(sem_relu, 1)
                    # layer-2 reduction, t=1 half: in-place Copy with
                    # accum_out sums along the free axis (t=0 runs on DVE);
                    # muls are ordered t1-first so this starts after mul #3
                    scalar.wait_ge(sem_mul, 4 * g + 3)
                    scalar.activation(
                        scr[:, 1, :], scr[:, 1, :], Act.Copy,
                        accum_out=logits[:, 2 * g + 1:2 * g + 2]).then_inc(
                            sem_l2, 1)

            @block.tensor
            def _(tensor):
                # ungated warmup on SBUF garbage: results land in psj and
                # are never read.  The PE clock reaches full speed ~5-6us
                # after the start of CONTINUOUS activity and a gap resets
                # the ramp, so the burst is sized to bridge from engine
                # boot (~6.3us) to vt/b1 arrival with no idle window.
                # full-width (M=128) dummies draw maximal PE power to pull
                # the DVFS boost forward
                for _ in range(N_WARMUP_512):
                    tensor.matmul(psj[:], wz[:, 0:128], wz[:],
                                  start=True, stop=True, skip_group_check=True)
                for _ in range(N_WARMUP_256):
                    tensor.matmul(psj[:, 0:256], wz[:, 0:128], wz[:, 0:256],
                                  start=True, stop=True, skip_group_check=True)
                tensor.wait_ge(sem_init, 2)  # ones row of vt
                tensor.wait_ge(sem_b1, 16)

                def bias(g):
                    # starts the accumulation group so the expert's tail
                    # ends on its last W1 k-tile's matmul
                    for lo, hi in regions:
                        tensor.matmul(
                            ps[g % 2][:, lo:hi], vt[0:1, KT, g, :],
                            b1[0:1, g, lo:hi], start=True, stop=False,
                            skip_group_check=True)

                gch = [[c for c in chunks if c[1] == g] for g in range(G)]
                bias(0)
                for g in range(G):
                    for ci, (i, _, k0, kc) in enumerate(gch[g]):
                        if g == 0 and k0 == 0:
                            tensor.wait_ge(sem_vta, 16)
                        if g == 0 and k0 == 3:
                            tensor.wait_ge(sem_vtb, 16)
                        if ci == len(gch[g]) - 1 and g + 1 < G:
                            # hoist the next expert's bias into this k-
                            # stream, BEFORE the last slab's wait so the
                            # bias matmuls execute during any supply stall:
                            # by now relu of g-1 has long freed the other
                            # PSUM slot (regions alternate per k so
                            # consecutive matmuls hit different PSUM banks)
                            tensor.wait_ge(sem_relu, g)
                            bias(g + 1)
                        tensor.wait_ge(w1sem[i], 16)
                        for kk in range(kc):
                            k = k0 + kk
                            for lo, hi in regions:
                                inst = tensor.matmul(
                                    ps[g % 2][:, lo:hi],
                                    vt[:, k, g, :],
                                    w1[:, g, k, lo:hi],
                                    start=False, stop=(k == KT - 1),
                                )
                    # fires at retirement: scalar's relu waits on this
                    inst.then_inc(sem_chunk, 1)

            @block.vector
            def _(vector):
                vector.memset(zb[:], 0.0).then_inc(sem_init, 1)
                vector.memset(vt[0:1, KT, :, :], 1.0).then_inc(sem_init, 1)
                vector.memset(o1[:, 0, H:H + 1], 1.0)
                vector.memset(o1[:, 1, H:H + 1], 1.0)
                vector.wait_ge(sem_w2, 16)
                for g in range(G):
                    # lo-region muls overlap ACT's hi-region relu
                    vector.wait_ge(sem_reluA, g + 1)
                    if g >= 1:
                        # scr slots free once both g-1 reductions are done
                        vector.wait_ge(sem_l2, 2 * g)
                    for t in (1, 0):
                        vector.tensor_mul(
                            scr[:, t, 0:512], o1[:, g % 2, 0:512],
                            w2[:, g, t, 0:512]).then_inc(sem_mul, 1)
                    vector.wait_ge(sem_relu, g + 1)
                    for t in (1, 0):
                        vector.tensor_mul(
                            scr[:, t, 512:H + 1], o1[:, g % 2, 512:H + 1],
                            w2[:, g, t, 512:H + 1]).then_inc(sem_mul, 1)
                    # t=0 reduction on DVE in parallel with ACT's t=1 half;
                    # the sem_mul wait makes it completion-tied to the muls
                    vector.wait_ge(sem_mul, 4 * g + 4)
                    vector.reduce_sum(
                        logits[:, 2 * g:2 * g + 1], scr[:, 0, :],
                        axis=mybir.AxisListType.X).then_inc(sem_l2, 1)

    # Strip the ctor-emitted all-engine barrier (incl. a ~3us GpSimd
    # dge-drain) from `main`: nothing reads the const APs it fences, and
    # every cross-engine dependency in this kernel is explicitly
    # semaphored, so the input streams can start as soon as engines boot.
    main_bb = nc.m.functions[0].blocks[0]

    def _is_ctor_barrier(inst):
        if type(inst).__name__ == "InstDrain":
            return True
        si = inst.sync_info
        if si is None:
            return False
        names = [u.ant_name or "" for u in (si.on_update or [])]
        names += [getattr(w, "ant_name", "") or "" for w in (si.on_wait or [])]
        return any(n.startswith("barrier_") for n in names)

    kept = [i for i in main_bb.instructions if not _is_ctor_barrier(i)]
    if len(kept) != len(main_bb.instructions):
        main_bb.instructions[:] = kept

    nc.compile()
    return nc


def _route(ids: np.ndarray, n_experts: int):
    """Group sample indices by expert; split groups >64; pad count to 8k."""
    CAP = 64
    groups = []
    for e in range(n_experts):
        idx = np.nonzero(ids == e)[0]
        if len(idx) <= CAP:
            groups.append((e, idx))
        else:
            for j in range(0, len(idx), CAP):
                groups.append((e, idx[j:j + CAP]))
    while len(groups) % N_CORES:
        groups.append((0, np.empty(0, np.int64)))
    G = len(groups) // N_CORES
    C = max(max((len(i) for _, i in groups)), 1)
    return groups, G, C


def _run(inputs: dict, trace: bool = False, **run_kwargs):
    v_X = np.asarray(inputs["v_X"], dtype=np.float32)
    v_H = np.asarray(inputs["v_H"], dtype=np.float32)
    ids = np.asarray(inputs["aspect_ids"]).astype(np.int64)
    W1 = np.asarray(inputs["W1_embs"], dtype=np.float32)
    b1 = np.asarray(inputs["b1_embs"], dtype=np.float32)
    W2 = np.asarray(inputs["W2_embs"], dtype=np.float32)
    b2 = np.asarray(inputs["b2_embs"], dtype=np.float32)

    B = v_X.shape[0]
    A = W1.shape[0]
    V = np.concatenate([v_X, v_H], axis=1)  # (B, D)
    assert V.shape[1] == D and b1.shape[1] == H

    groups, G, C = _route(ids, A)

    key = (G, C)
    if key not in _graph_cache:
        _graph_cache[key] = _build(G, C)
    nc = _graph_cache[key]

    bf16 = ml_dtypes.bfloat16
    f8 = ml_dtypes.float8_e3m4
    sw = np.float32(W1_SCALE)
    in_maps = []
    for c in range(N_CORES):
        cg = groups[c * G:(c + 1) * G]
        # [g, p, k, h] layout, x128 into e3m4 (amax*128 = 13.9 < 15.5 max)
        w1c = np.stack([(W1[e] * sw).reshape(KT, 128, H).transpose(1, 0, 2)
                        for e, _ in cg]).astype(f8)
        vtc = np.zeros((128, KT + 1, G, C), dtype=bf16)
        w2c = np.zeros((C, G, 2, H + 1), dtype=bf16)
        b1c = (np.stack([b1[e] for e, _ in cg])[None] * sw).astype(bf16)
        for g, (e, idx) in enumerate(cg):
            n = len(idx)
            if n:
                # V[idx].T: (D, n) -> (KT, 128, n) -> [p, k, c]
                vtc[:, :KT, g, :n] = (
                    V[idx].T.reshape(KT, 128, n).transpose(1, 0, 2).astype(bf16))
            w2r = W2[e].reshape(H, 2) / sw  # undo the W1 scale after relu
            w2c[:, g, 0, :H] = w2r[:, 0].astype(bf16)
            w2c[:, g, 1, :H] = w2r[:, 1].astype(bf16)
            w2c[:, g, 0, H] = b2[e, 0]
            w2c[:, g, 1, H] = b2[e, 1]
        in_maps.append({
            "w1": np.ascontiguousarray(w1c),
            "vt": np.ascontiguousarray(vtc),
            "b1r": np.ascontiguousarray(b1c),
            "w2e": np.ascontiguousarray(w2c),
        })

    res = run_bass_kernel_spmd(nc, in_maps, core_ids=list(range(N_CORES)),
                               trace=trace, **run_kwargs)

    logits = np.zeros((B, 2), dtype=np.float32)
    for c in range(N_CORES):
        out_c = res.results[c]["out"]  # (C, 2G)
        for g, (e, idx) in enumerate(groups[c * G:(c + 1) * G]):
            n = len(idx)
            if n:
                logits[idx] = out_c[:n, 2 * g:2 * g + 2]
    return logits, res


def kernel(**inputs) -> np.ndarray:
    logits, _ = _run(inputs, trace=False)
    return logits


# revision 57
# speedup vs baseline: 1.1496x; 1.0073x over previous
"""Trainium2 Bass kernel for per-sample expert-routed 2-layer MLP (MoE routing).

Problem: logits[b] = relu(V[b] @ W1[id[b]] + b1[id[b]]) @ W2[id[b]] + b2[id[b]]
  V = concat(v_X, v_H): (256, 1536), 32 experts, W1 per expert (1536, 768).

Strategy (expert parallel over 8 NeuronCores, raw bacc pipeline):
  - Host routes samples to experts, assigns 4 experts per core, and casts
    W1 to float8_e3m4 (x128 scale, folded out of b1/W2 on host), so each
    expert's 1.18 MB streams HBM->SBUF exactly once chip-wide at one byte
    per element -- half the bf16 traffic.  The whole per-core W1 (4.7 MB
    = 36.9 KB/partition) is SBUF-resident: no ring, no recycle waits.
  - W1 slab DMAs issue from the sync engine (fine-grained for expert 0 so
    the PE starts early, 6-k-tile slabs after -- dma_start issue costs
    ~750ns + 5ns/descriptor of sequencer time, so big slabs keep the
    stream issue-rate above the queue bandwidth).  vt/b1/w2 issue from
    GpSimd, whose Q7 boots earliest.
  - The TensorEngine chases the stream with V^T stationary (bf16; matmul
    operands mix dtypes in normal mode).  PSUM regions 0:512/512:768
    alternate per k-tile: consecutive matmuls into the same PSUM bank
    stall ~330ns, alternating banks run back-to-back.  A dummy-matmul
    burst at boot starts the PE clock ramp before real data lands.
  - Bias starts each PSUM accumulation group as a K=1 matmul against an
    all-ones row.
  - Layer 2 (768 -> 2): four fused multiply-reduce passes on the DVE
    (tensor_tensor_reduce, region-chained via the init-value AP; b2 via a
    ones column), overlapped with the scalar engine's two-region relu.
  - Outputs (capacity-padded per-expert logits) are scattered on host.
"""

from contextlib import ExitStack

import ml_dtypes
import numpy as np

import concourse.bacc as bacc
import concourse.mybir as mybir
from concourse.bass_utils import run_bass_kernel_spmd

N_CORES = 8
KT = 12          # K tiles of 128 over D=1536
D = 1536
H = 768
W1_SCALE = 128.0
N_WARMUP_512 = 5
N_WARMUP_256 = 3
# k-tiles per W1 slab DMA, per expert-group.  Every slab costs 128
# descriptors (~0.1-0.25us of queue processing each, plus ~0.8us of
# sequencer issue), so k-tile SUPPLY RATE scales with slab size: the
# head matches the mid-clock PE rate, late experts ship whole to cut
# total descriptor count (and so total queue time).
K_CHUNKS = ((1, 2, 3, 3, 3), (6, 6), (6, 6), (6, 6))

_graph_cache = {}


def _build(G: int, C: int):
    """Build the SPMD graph: G expert-groups per core, capacity C samples."""
    dt = mybir.dt
    Act = mybir.ActivationFunctionType
    Alu = mybir.AluOpType

    nc = bacc.Bacc("TRN2", target_bir_lowering=False, debug=False,
                   enable_asserts=False, monotonic_sem_count=0)

    chunks = []  # (global_id, g, k0, kc)
    for g in range(G):
        for kc in K_CHUNKS[g] if g < len(K_CHUNKS) else K_CHUNKS[-1]:
            k0 = sum(c[3] for c in chunks if c[1] == g)
            chunks.append((len(chunks), g, k0, kc))
    NSLAB = len(chunks)

    w1d = nc.dram_tensor("w1", [G, 128, KT, H], dt.float8e3, kind="ExternalInput")
    vtd = nc.dram_tensor("vt", [128, KT + 1, G, C], dt.bfloat16, kind="ExternalInput")
    b1d = nc.dram_tensor("b1r", [1, G, H], dt.bfloat16, kind="ExternalInput")
    w2d = nc.dram_tensor("w2e", [C, G, 2, H + 1], dt.bfloat16, kind="ExternalInput")
    outd = nc.dram_tensor("out", [C, 2 * G], dt.float32, kind="ExternalOutput")

    regions = ((0, 512), (512, H))

    with ExitStack() as ctx:
        en = ctx.enter_context
        wz = en(nc.sbuf_tensor("wz", [128, 512], dt.bfloat16))
        w1 = en(nc.sbuf_tensor("w1_sb", [128, G, KT, H], dt.float8e3))
        vt = en(nc.sbuf_tensor("vt_sb", [128, KT + 1, G, C], dt.bfloat16))
        b1 = en(nc.sbuf_tensor("b1_sb", [1, G, H], dt.bfloat16))
        w2 = en(nc.sbuf_tensor("w2_sb", [C, G, 2, H + 1], dt.bfloat16))
        o1 = en(nc.sbuf_tensor("o1", [C, 2, H + 1], dt.bfloat16))
        scr = en(nc.sbuf_tensor("scr", [C, 2, H + 1], dt.bfloat16))
        logits = en(nc.sbuf_tensor("logits", [C, 2 * G], dt.float32))
        part = en(nc.sbuf_tensor("part", [C, 2], dt.float32))
        zb = en(nc.sbuf_tensor("zb", [C, 1], dt.float32))
        ps = [en(nc.psum_tensor(f"ps{i}", [C, H], dt.float32)) for i in range(2)]
        psj = en(nc.psum_tensor("psj", [128, 512], dt.float32))

        sem_init = en(nc.semaphore("sem_init"))
        # one semaphore per concurrently-in-flight DMA stream: a wait on a
        # semaphore fed by several unordered DMA completions is unsound
        sem_vta = en(nc.semaphore("sem_vta"))
        sem_vtb = en(nc.semaphore("sem_vtb"))
        sem_b1 = en(nc.semaphore("sem_b1"))
        sem_w2 = en(nc.semaphore("sem_w2"))
        w1sem = [en(nc.semaphore(f"sem_w1s{i}")) for i in range(NSLAB)]
        sem_chunk = en(nc.semaphore("sem_chunk"))
        sem_pe = en(nc.semaphore("sem_pe"))
        sem_relu = en(nc.semaphore("sem_relu"))
        sem_reluA = en(nc.semaphore("sem_reluA"))
        sem_mul = en(nc.semaphore("sem_mul"))
        sem_l2 = en(nc.semaphore("sem_l2"))
        sem_out = en(nc.semaphore("sem_out"))

        with nc.Block(no_gpsimd_drain=True) as block:

            @block.gpsimd
            def _(gpsimd):
                # W1 slab stream: issue-and-forget, consumption order
                for i, g, k0, kc in chunks:
                    gpsimd.dma_start(
                        w1[:, g, k0:k0 + kc, :],
                        w1d[g, :, k0:k0 + kc, :],
                    ).then_inc(w1sem[i], 16)

            @block.sync
            def _(sync):
                # sync's program starts earliest (~6.2us).  vt is split so
                # the PE's first k-tiles don't wait for the whole batch;
                # the ones row (k index KT) is memset on-device instead of
                # transferred, so the bias matmul only needs b1.
                sync.dma_start(b1[:], b1d[:]).then_inc(sem_b1, 16)
                sync.dma_start(vt[:, 0:3], vtd[:, 0:3]).then_inc(sem_vta, 16)
                sync.dma_start(vt[:, 3:KT], vtd[:, 3:KT]).then_inc(sem_vtb, 16)
                sync.dma_start(w2[:], w2d[:]).then_inc(sem_w2, 16)
                sync.wait_ge(sem_l2, 2 * G)
                sync.dma_start(outd[:], logits[:]).then_inc(sem_out, 16)
                # no final receipt wait: the SP drain at block exit flushes
                # the HWDGE queue before the NEFF retires

            @block.scalar
            def _(scalar):
                scalar.wait_ge(sem_init, 1)  # zb ready
                for g in range(G):
                    scalar.wait_ge(sem_chunk, g + 1)
                    if g >= 2:
                        # o1 slot free once DVE finished expert g-2's ttrs
                        scalar.wait_ge(sem_mul, 4 * g - 4)
                    scalar.activation(
                        o1[:, g % 2, 0:512], ps[g % 2][:, 0:512],
                        Act.Relu, bias=zb[:]).then_inc(sem_reluA, 1)
                    scalar.activation(
                        o1[:, g % 2, 512:H], ps[g % 2][:, 512:H],
                        Act.Relu, bias=zb[:]).then_inc(sem_relu, 1)
                    # layer-2 reduction, t=1 half: in-place Copy with
                    # accum_out sums along the free axis (t=0 runs on DVE);
                    # muls are ordered t1-first so this starts after mul #3
                    scalar.wait_ge(sem_mul, 4 * g + 3)
                    scalar.activation(
                        scr[:, 1, :], scr[:, 1, :], Act.Copy,
                        accum_out=logits[:, 2 * g + 1:2 * g + 2]).then_inc(
                            sem_l2, 1)

            @block.tensor
            def _(tensor):
                # ungated warmup on SBUF garbage: results land in psj and
                # are never read.  The PE clock reaches full speed ~5-6us
                # after the start of CONTINUOUS activity and a gap resets
                # the ramp, so the burst is sized to bridge from engine
                # boot (~6.3us) to vt/b1 arrival with no idle window.
                # full-width (M=128) dummies draw maximal PE power to pull
                # the DVFS boost forward
                for _ in range(N_WARMUP_512):
                    tensor.matmul(psj[:], wz[:, 0:128], wz[:],
                                  start=True, stop=True, skip_group_check=True)
                for _ in range(N_WARMUP_256):
                    tensor.matmul(psj[:, 0:256], wz[:, 0:128], wz[:, 0:256],
                                  start=True, stop=True, skip_group_check=True)
                tensor.wait_ge(sem_init, 2)  # ones row of vt
                tensor.wait_ge(sem_b1, 16)

                def bias(g):
                    # starts the accumulation group so the expert's tail
                    # ends on its last W1 k-tile's matmul
                    for lo, hi in regions:
                        tensor.matmul(
                            ps[g % 2][:, lo:hi], vt[0:1, KT, g, :],
                            b1[0:1, g, lo:hi], start=True, stop=False,
                            skip_group_check=True)

                gch = [[c for c in chunks if c[1] == g] for g in range(G)]
                bias(0)
                for g in range(G):
                    for ci, (i, _, k0, kc) in enumerate(gch[g]):
                        if g == 0 and k0 == 0:
                            tensor.wait_ge(sem_vta, 16)
                        if g == 0 and k0 == 3:
                            tensor.wait_ge(sem_vtb, 16)
                        if ci == len(gch[g]) - 1 and g + 1 < G:
                            # hoist the next expert's bias into this k-
                            # stream, BEFORE the last slab's wait so the
                            # bias matmuls execute during any supply stall:
                            # by now relu of g-1 has long freed the other
                            # PSUM slot (regions alternate per k so
                            # consecutive matmuls hit different PSUM banks)
                            tensor.wait_ge(sem_relu, g)
                            bias(g + 1)
                        tensor.wait_ge(w1sem[i], 16)
                        for kk in range(kc):
                            k = k0 + kk
                            for lo, hi in regions:
                                inst = tensor.matmul(
                                    ps[g % 2][:, lo:hi],
                                    vt[:, k, g, :],
                                    w1[:, g, k, lo:hi],
                                    start=False, stop=(k == KT - 1),
                                )
                    # fires at retirement: scalar's relu waits on this
                    inst.then_inc(sem_chunk, 1)

            @block.vector
            def _(vector):
                vector.memset(zb[:], 0.0).then_inc(sem_init, 1)
                vector.memset(vt[0:1, KT, :, :], 1.0).then_inc(sem_init, 1)
                vector.memset(o1[:, 0, H:H + 1], 1.0)
                vector.memset(o1[:, 1, H:H + 1], 1.0)
                vector.wait_ge(sem_w2, 16)
                for g in range(G):
                    # lo-region muls overlap ACT's hi-region relu
                    vector.wait_ge(sem_reluA, g + 1)
                    if g >= 1:
                        # scr slots free once both g-1 reductions are done
                        vector.wait_ge(sem_l2, 2 * g)
                    for t in (1, 0):
                        vector.tensor_mul(
                            scr[:, t, 0:512], o1[:, g % 2, 0:512],
                            w2[:, g, t, 0:512]).then_inc(sem_mul, 1)
                    vector.wait_ge(sem_relu, g + 1)
                    for t in (1, 0):
                        vector.tensor_mul(
                            scr[:, t, 512:H + 1], o1[:, g % 2, 512:H + 1],
                            w2[:, g, t, 512:H + 1]).then_inc(sem_mul, 1)
                    # t=0 reduction on DVE in parallel with ACT's t=1 half;
                    # the sem_mul wait makes it completion-tied to the muls
                    vector.wait_ge(sem_mul, 4 * g + 4)
                    vector.reduce_sum(
                        logits[:, 2 * g:2 * g + 1], scr[:, 0, :],
                        axis=mybir.AxisListType.X).then_inc(sem_l2, 1)

    # Strip the ctor-emitted all-engine barrier (incl. a ~3us GpSimd
    # dge-drain) from `main`: nothing reads the const APs it fences, and
    # every cross-engine dependency in this kernel is explicitly
    # semaphored, so the input streams can start as soon as engines boot.
    main_bb = nc.m.functions[0].blocks[0]

    def _is_ctor_barrier(inst):
        if type(inst).__name__ == "InstDrain":
            return True
        si = inst.sync_info
        if si is None:
            return False
        names = [u.ant_name or "" for u in (si.on_update or [])]
        names += [getattr(w, "ant_name", "") or "" for w in (si.on_wait or [])]
        return any(n.startswith("barrier_") for n in names)

    kept = [i for i in main_bb.instructions if not _is_ctor_barrier(i)]
    if len(kept) != len(main_bb.instructions):
        main_bb.instructions[:] = kept

    nc.compile()
    return nc


def _route(ids: np.ndarray, n_experts: int):
    """Group sample indices by expert; split groups >64; pad count to 8k."""
    CAP = 64
    groups = []
    for e in range(n_experts):
        idx = np.nonzero(ids == e)[0]
        if len(idx) <= CAP:
            groups.append((e, idx))
        else:
            for j in range(0, len(idx), CAP):
                groups.append((e, idx[j:j + CAP]))
    while len(groups) % N_CORES:
        groups.append((0, np.empty(0, np.int64)))
    G = len(groups) // N_CORES
    C = max(max((len(i) for _, i in groups)), 1)
    return groups, G, C


def _run(inputs: dict, trace: bool = False, **run_kwargs):
    v_X = np.asarray(inputs["v_X"], dtype=np.float32)
    v_H = np.asarray(inputs["v_H"], dtype=np.float32)
    ids = np.asarray(inputs["aspect_ids"]).astype(np.int64)
    W1 = np.asarray(inputs["W1_embs"], dtype=np.float32)
    b1 = np.asarray(inputs["b1_embs"], dtype=np.float32)
    W2 = np.asarray(inputs["W2_embs"], dtype=np.float32)
    b2 = np.asarray(inputs["b2_embs"], dtype=np.float32)

    B = v_X.shape[0]
    A = W1.shape[0]
    V = np.concatenate([v_X, v_H], axis=1)  # (B, D)
    assert V.shape[1] == D and b1.shape[1] == H

    groups, G, C = _route(ids, A)

    key = (G, C)
    if key not in _graph_cache:
        _graph_cache[key] = _build(G, C)
    nc = _graph_cache[key]

    bf16 = ml_dtypes.bfloat16
    f8 = ml_dtypes.float8_e3m4
    sw = np.float32(W1_SCALE)
    in_maps = []
    for c in range(N_CORES):
        cg = groups[c * G:(c + 1) * G]
        # [g, p, k, h] layout, x128 into e3m4 (amax*128 = 13.9 < 15.5 max)
        w1c = np.stack([(W1[e] * sw).reshape(KT, 128, H).transpose(1, 0, 2)
                        for e, _ in cg]).astype(f8)
        vtc = np.zeros((128, KT + 1, G, C), dtype=bf16)
        w2c = np.zeros((C, G, 2, H + 1), dtype=bf16)
        b1c = (np.stack([b1[e] for e, _ in cg])[None] * sw).astype(bf16)
        for g, (e, idx) in enumerate(cg):
            n = len(idx)
            if n:
                # V[idx].T: (D, n) -> (KT, 128, n) -> [p, k, c]
                vtc[:, :KT, g, :n] = (
                    V[idx].T.reshape(KT, 128, n).transpose(1, 0, 2).astype(bf16))
            w2r = W2[e].reshape(H, 2) / sw  # undo the W1 scale after relu
            w2c[:, g, 0, :H] = w2r[:, 0].astype(bf16)
            w2c[:, g, 1, :H] = w2r[:, 1].astype(bf16)
            w2c[:, g, 0, H] = b2[e, 0]
            w2c[:, g, 1, H] = b2[e, 1]
        in_maps.append({
            "w1": np.ascontiguousarray(w1c),
            "vt": np.ascontiguousarray(vtc),
            "b1r": np.ascontiguousarray(b1c),
            "w2e": np.ascontiguousarray(w2c),
        })

    res = run_bass_kernel_spmd(nc, in_maps, core_ids=list(range(N_CORES)),
                               trace=trace, **run_kwargs)

    logits = np.zeros((B, 2), dtype=np.float32)
    for c in range(N_CORES):
        out_c = res.results[c]["out"]  # (C, 2G)
        for g, (e, idx) in enumerate(groups[c * G:(c + 1) * G]):
            n = len(idx)
            if n:
                logits[idx] = out_c[:n, 2 * g:2 * g + 2]
    return logits, res


def kernel(**inputs) -> np.ndarray:
    logits, _ = _run(inputs, trace=False)
    return logits


# revision 58
# speedup vs baseline: 1.1498x; 1.0001x over previous
"""Trainium2 Bass kernel for per-sample expert-routed 2-layer MLP (MoE routing).

Problem: logits[b] = relu(V[b] @ W1[id[b]] + b1[id[b]]) @ W2[id[b]] + b2[id[b]]
  V = concat(v_X, v_H): (256, 1536), 32 experts, W1 per expert (1536, 768).

Strategy (expert parallel over 8 NeuronCores, raw bacc pipeline):
  - Host routes samples to experts, assigns 4 experts per core, and casts
    W1 to float8_e3m4 (x128 scale, folded out of b1/W2 on host), so each
    expert's 1.18 MB streams HBM->SBUF exactly once chip-wide at one byte
    per element -- half the bf16 traffic.  The whole per-core W1 (4.7 MB
    = 36.9 KB/partition) is SBUF-resident: no ring, no recycle waits.
  - W1 slab DMAs issue from the sync engine (fine-grained for expert 0 so
    the PE starts early, 6-k-tile slabs after -- dma_start issue costs
    ~750ns + 5ns/descriptor of sequencer time, so big slabs keep the
    stream issue-rate above the queue bandwidth).  vt/b1/w2 issue from
    GpSimd, whose Q7 boots earliest.
  - The TensorEngine chases the stream with V^T stationary (bf16; matmul
    operands mix dtypes in normal mode).  PSUM regions 0:512/512:768
    alternate per k-tile: consecutive matmuls into the same PSUM bank
    stall ~330ns, alternating banks run back-to-back.  A dummy-matmul
    burst at boot starts the PE clock ramp before real data lands.
  - Bias starts each PSUM accumulation group as a K=1 matmul against an
    all-ones row, hoisted into the PREVIOUS expert's k-stream (before its
    last slab wait) so expert boundaries are gapless back-to-back matmuls.
  - Layer 2 (768 -> 2): the vector engine forms bf16 products against
    host-broadcast W2 columns (b2 via a ones column); the scalar engine
    reduces t=1 with an accumulating in-place Copy while the DVE reduces
    t=0, both overlapped with the next expert's matmuls.
  - Outputs (capacity-padded per-expert logits) are scattered on host.
    NOTE for future work: seeding the bias into PSUM from the ACT engine
    (removing the bias matmuls) measured ~1.5us faster but produced a
    NON-DETERMINISTIC rel-err drift (1.63-1.88e-2 vs the stable 1.551e-2)
    that survived both a completion-fence and PSUM bank-alignment fix;
    reverted.  Do not re-attempt without solving that race.
"""

from contextlib import ExitStack

import ml_dtypes
import numpy as np

import concourse.bacc as bacc
import concourse.mybir as mybir
from concourse.bass_utils import run_bass_kernel_spmd

N_CORES = 8
KT = 12          # K tiles of 128 over D=1536
D = 1536
H = 768
W1_SCALE = 128.0
N_WARMUP_512 = 5
N_WARMUP_256 = 3
# k-tiles per W1 slab DMA, per expert-group.  Every slab costs 128
# descriptors (~0.1-0.25us of queue processing each, plus ~0.8us of
# sequencer issue), so k-tile SUPPLY RATE scales with slab size: the
# head matches the mid-clock PE rate, late experts ship whole to cut
# total descriptor count (and so total queue time).
K_CHUNKS = ((1, 2, 3, 3, 3), (6, 6), (6, 6), (6, 6))

_graph_cache = {}


def _build(G: int, C: int):
    """Build the SPMD graph: G expert-groups per core, capacity C samples."""
    dt = mybir.dt
    Act = mybir.ActivationFunctionType
    Alu = mybir.AluOpType

    nc = bacc.Bacc("TRN2", target_bir_lowering=False, debug=False,
                   enable_asserts=False, monotonic_sem_count=0)

    chunks = []  # (global_id, g, k0, kc)
    for g in range(G):
        for kc in K_CHUNKS[g] if g < len(K_CHUNKS) else K_CHUNKS[-1]:
            k0 = sum(c[3] for c in chunks if c[1] == g)
            chunks.append((len(chunks), g, k0, kc))
    NSLAB = len(chunks)

    w1d = nc.dram_tensor("w1", [G, 128, KT, H], dt.float8e3, kind="ExternalInput")
    vtd = nc.dram_tensor("vt", [128, KT + 1, G, C], dt.bfloat16, kind="ExternalInput")
    b1d = nc.dram_tensor("b1r", [1, G, H], dt.bfloat16, kind="ExternalInput")
    w2d = nc.dram_tensor("w2e", [C, G, 2, H + 1], dt.bfloat16, kind="ExternalInput")
    outd = nc.dram_tensor("out", [C, 2 * G], dt.float32, kind="ExternalOutput")

    regions = ((0, 512), (512, H))

    with ExitStack() as ctx:
        en = ctx.enter_context
        wz = en(nc.sbuf_tensor("wz", [128, 512], dt.bfloat16))
        w1 = en(nc.sbuf_tensor("w1_sb", [128, G, KT, H], dt.float8e3))
        vt = en(nc.sbuf_tensor("vt_sb", [128, KT + 1, G, C], dt.bfloat16))
        b1 = en(nc.sbuf_tensor("b1_sb", [1, G, H], dt.bfloat16))
        w2 = en(nc.sbuf_tensor("w2_sb", [C, G, 2, H + 1], dt.bfloat16))
        o1 = en(nc.sbuf_tensor("o1", [C, 2, H + 1], dt.bfloat16))
        scr = en(nc.sbuf_tensor("scr", [C, 2, H + 1], dt.bfloat16))
        logits = en(nc.sbuf_tensor("logits", [C, 2 * G], dt.float32))
        part = en(nc.sbuf_tensor("part", [C, 2], dt.float32))
        zb = en(nc.sbuf_tensor("zb", [C, 1], dt.float32))
        ps = [en(nc.psum_tensor(f"ps{i}", [C, H], dt.float32)) for i in range(2)]
        psj = en(nc.psum_tensor("psj", [128, 512], dt.float32))

        sem_init = en(nc.semaphore("sem_init"))
        # one semaphore per concurrently-in-flight DMA stream: a wait on a
        # semaphore fed by several unordered DMA completions is unsound
        sem_vta = en(nc.semaphore("sem_vta"))
        sem_vtb = en(nc.semaphore("sem_vtb"))
        sem_b1 = en(nc.semaphore("sem_b1"))
        sem_w2 = en(nc.semaphore("sem_w2"))
        w1sem = [en(nc.semaphore(f"sem_w1s{i}")) for i in range(NSLAB)]
        sem_chunk = en(nc.semaphore("sem_chunk"))
        sem_pe = en(nc.semaphore("sem_pe"))
        sem_relu = en(nc.semaphore("sem_relu"))
        sem_reluA = en(nc.semaphore("sem_reluA"))
        sem_mul = en(nc.semaphore("sem_mul"))
        sem_l2 = en(nc.semaphore("sem_l2"))
        sem_out = en(nc.semaphore("sem_out"))

        with nc.Block(no_gpsimd_drain=True) as block:

            @block.gpsimd
            def _(gpsimd):
                # W1 slab stream: issue-and-forget, consumption order
                for i, g, k0, kc in chunks:
                    gpsimd.dma_start(
                        w1[:, g, k0:k0 + kc, :],
                        w1d[g, :, k0:k0 + kc, :],
                    ).then_inc(w1sem[i], 16)

            @block.sync
            def _(sync):
                # sync's program starts earliest (~6.2us).  vt is split so
                # the PE's first k-tiles don't wait for the whole batch;
                # the ones row (k index KT) is memset on-device instead of
                # transferred, so the bias matmul only needs b1.
                sync.dma_start(b1[:], b1d[:]).then_inc(sem_b1, 16)
                sync.dma_start(vt[:, 0:3], vtd[:, 0:3]).then_inc(sem_vta, 16)
                sync.dma_start(vt[:, 3:KT], vtd[:, 3:KT]).then_inc(sem_vtb, 16)
                sync.dma_start(w2[:], w2d[:]).then_inc(sem_w2, 16)
                sync.wait_ge(sem_l2, 2 * G)
                sync.dma_start(outd[:], logits[:]).then_inc(sem_out, 16)
                # no final receipt wait: the SP drain at block exit flushes
                # the HWDGE queue before the NEFF retires

            @block.scalar
            def _(scalar):
                scalar.wait_ge(sem_init, 1)  # zb ready
                for g in range(G):
                    scalar.wait_ge(sem_chunk, g + 1)
                    if g >= 2:
                        # o1 slot free once DVE finished expert g-2's ttrs
                        scalar.wait_ge(sem_mul, 4 * g - 4)
                    scalar.activation(
                        o1[:, g % 2, 0:512], ps[g % 2][:, 0:512],
                        Act.Relu, bias=zb[:]).then_inc(sem_reluA, 1)
                    scalar.activation(
                        o1[:, g % 2, 512:H], ps[g % 2][:, 512:H],
                        Act.Relu, bias=zb[:]).then_inc(sem_relu, 1)
                    # layer-2 reduction, t=1 half: in-place Copy with
                    # accum_out sums along the free axis (t=0 runs on DVE);
                    # muls are ordered t1-first so this starts after mul #3
                    scalar.wait_ge(sem_mul, 4 * g + 3)
                    scalar.activation(
                        scr[:, 1, :], scr[:, 1, :], Act.Copy,
                        accum_out=logits[:, 2 * g + 1:2 * g + 2]).then_inc(
                            sem_l2, 1)

            @block.tensor
            def _(tensor):
                # ungated warmup on SBUF garbage: results land in psj and
                # are never read.  The PE clock reaches full speed ~5-6us
                # after the start of CONTINUOUS activity and a gap resets
                # the ramp, so the burst is sized to bridge from engine
                # boot (~6.3us) to vt/b1 arrival with no idle window.
                # full-width (M=128) dummies draw maximal PE power to pull
                # the DVFS boost forward
                for _ in range(N_WARMUP_512):
                    tensor.matmul(psj[:], wz[:, 0:128], wz[:],
                                  start=True, stop=True, skip_group_check=True)
                for _ in range(N_WARMUP_256):
                    tensor.matmul(psj[:, 0:256], wz[:, 0:128], wz[:, 0:256],
                                  start=True, stop=True, skip_group_check=True)
                tensor.wait_ge(sem_init, 2)  # ones row of vt
                tensor.wait_ge(sem_b1, 16)

                def bias(g):
                    # starts the accumulation group so the expert's tail
                    # ends on its last W1 k-tile's matmul
                    for lo, hi in regions:
                        tensor.matmul(
                            ps[g % 2][:, lo:hi], vt[0:1, KT, g, :],
                            b1[0:1, g, lo:hi], start=True, stop=False,
                            skip_group_check=True)

                gch = [[c for c in chunks if c[1] == g] for g in range(G)]
                bias(0)
                for g in range(G):
                    for ci, (i, _, k0, kc) in enumerate(gch[g]):
                        if g == 0 and k0 == 0:
                            tensor.wait_ge(sem_vta, 16)
                        if g == 0 and k0 == 3:
                            tensor.wait_ge(sem_vtb, 16)
                        if ci == len(gch[g]) - 1 and g + 1 < G:
                            # hoist the next expert's bias into this k-
                            # stream, BEFORE the last slab's wait so the
                            # bias matmuls execute during any supply stall:
                            # by now relu of g-1 has long freed the other
                            # PSUM slot (regions alternate per k so
                            # consecutive matmuls hit different PSUM banks)
                            tensor.wait_ge(sem_relu, g)
                            bias(g + 1)
                        tensor.wait_ge(w1sem[i], 16)
                        for kk in range(kc):
                            k = k0 + kk
                            for lo, hi in regions:
                                inst = tensor.matmul(
                                    ps[g % 2][:, lo:hi],
                                    vt[:, k, g, :],
                                    w1[:, g, k, lo:hi],
                                    start=False, stop=(k == KT - 1),
                                )
                    # fires at retirement: scalar's relu waits on this
                    inst.then_inc(sem_chunk, 1)

            @block.vector
            def _(vector):
                vector.memset(zb[:], 0.0).then_inc(sem_init, 1)
                vector.memset(vt[0:1, KT, :, :], 1.0).then_inc(sem_init, 1)
                vector.memset(o1[:, 0, H:H + 1], 1.0)
                vector.memset(o1[:, 1, H:H + 1], 1.0)
                vector.wait_ge(sem_w2, 16)
                for g in range(G):
                    # lo-region muls overlap ACT's hi-region relu
                    vector.wait_ge(sem_reluA, g + 1)
                    if g >= 1:
                        # scr slots free once both g-1 reductions are done
                        vector.wait_ge(sem_l2, 2 * g)
                    for t in (1, 0):
                        vector.tensor_mul(
                            scr[:, t, 0:512], o1[:, g % 2, 0:512],
                            w2[:, g, t, 0:512]).then_inc(sem_mul, 1)
                    vector.wait_ge(sem_relu, g + 1)
                    for t in (1, 0):
                        vector.tensor_mul(
                            scr[:, t, 512:H + 1], o1[:, g % 2, 512:H + 1],
                            w2[:, g, t, 512:H + 1]).then_inc(sem_mul, 1)
                    # t=0 reduction on DVE in parallel with ACT's t=1 half;
                    # the sem_mul wait makes it completion-tied to the muls
                    vector.wait_ge(sem_mul, 4 * g + 4)
                    vector.reduce_sum(
                        logits[:, 2 * g:2 * g + 1], scr[:, 0, :],
                        axis=mybir.AxisListType.X).then_inc(sem_l2, 1)

    # Strip the ctor-emitted all-engine barrier (incl. a ~3us GpSimd
    # dge-drain) from `main`: nothing reads the const APs it fences, and
    # every cross-engine dependency in this kernel is explicitly
    # semaphored, so the input streams can start as soon as engines boot.
    main_bb = nc.m.functions[0].blocks[0]

    def _is_ctor_barrier(inst):
        if type(inst).__name__ == "InstDrain":
            return True
        si = inst.sync_info
        if si is None:
            return False
        names = [u.ant_name or "" for u in (si.on_update or [])]
        names += [getattr(w, "ant_name", "") or "" for w in (si.on_wait or [])]
        return any(n.startswith("barrier_") for n in names)

    kept = [i for i in main_bb.instructions if not _is_ctor_barrier(i)]
    if len(kept) != len(main_bb.instructions):
        main_bb.instructions[:] = kept

    nc.compile()
    return nc


def _route(ids: np.ndarray, n_experts: int):
    """Group sample indices by expert; split groups >64; pad count to 8k."""
    CAP = 64
    groups = []
    for e in range(n_experts):
        idx = np.nonzero(ids == e)[0]
        if len(idx) <= CAP:
            groups.append((e, idx))
        else:
            for j in range(0, len(idx), CAP):
                groups.append((e, idx[j:j + CAP]))
    while len(groups) % N_CORES:
        groups.append((0, np.empty(0, np.int64)))
    G = len(groups) // N_CORES
    C = max(max((len(i) for _, i in groups)), 1)
    return groups, G, C


def _run(inputs: dict, trace: bool = False, **run_kwargs):
    v_X = np.asarray(inputs["v_X"], dtype=np.float32)
    v_H = np.asarray(inputs["v_H"], dtype=np.float32)
    ids = np.asarray(inputs["aspect_ids"]).astype(np.int64)
    W1 = np.asarray(inputs["W1_embs"], dtype=np.float32)
    b1 = np.asarray(inputs["b1_embs"], dtype=np.float32)
    W2 = np.asarray(inputs["W2_embs"], dtype=np.float32)
    b2 = np.asarray(inputs["b2_embs"], dtype=np.float32)

    B = v_X.shape[0]
    A = W1.shape[0]
    V = np.concatenate([v_X, v_H], axis=1)  # (B, D)
    assert V.shape[1] == D and b1.shape[1] == H

    groups, G, C = _route(ids, A)

    key = (G, C)
    if key not in _graph_cache:
        _graph_cache[key] = _build(G, C)
    nc = _graph_cache[key]

    bf16 = ml_dtypes.bfloat16
    f8 = ml_dtypes.float8_e3m4
    sw = np.float32(W1_SCALE)
    in_maps = []
    for c in range(N_CORES):
        cg = groups[c * G:(c + 1) * G]
        # [g, p, k, h] layout, x128 into e3m4 (amax*128 = 13.9 < 15.5 max)
        w1c = np.stack([(W1[e] * sw).reshape(KT, 128, H).transpose(1, 0, 2)
                        for e, _ in cg]).astype(f8)
        vtc = np.zeros((128, KT + 1, G, C), dtype=bf16)
        w2c = np.zeros((C, G, 2, H + 1), dtype=bf16)
        b1c = (np.stack([b1[e] for e, _ in cg])[None] * sw).astype(bf16)
        for g, (e, idx) in enumerate(cg):
            n = len(idx)
            if n:
                # V[idx].T: (D, n) -> (KT, 128, n) -> [p, k, c]
                vtc[:, :KT, g, :n] = (
                    V[idx].T.reshape(KT, 128, n).transpose(1, 0, 2).astype(bf16))
            w2r = W2[e].reshape(H, 2) / sw  # undo the W1 scale after relu
            w2c[:, g, 0, :H] = w2r[:, 0].astype(bf16)
            w2c[:, g, 1, :H] = w2r[:, 1].astype(bf16)
            w2c[:, g, 0, H] = b2[e, 0]
            w2c[:, g, 1, H] = b2[e, 1]
        in_maps.append({
            "w1": np.ascontiguousarray(w1c),
            "vt": np.ascontiguousarray(vtc),
            "b1r": np.ascontiguousarray(b1c),
            "w2e": np.ascontiguousarray(w2c),
        })

    res = run_bass_kernel_spmd(nc, in_maps, core_ids=list(range(N_CORES)),
                               trace=trace, **run_kwargs)

    logits = np.zeros((B, 2), dtype=np.float32)
    for c in range(N_CORES):
        out_c = res.results[c]["out"]  # (C, 2G)
        for g, (e, idx) in enumerate(groups[c * G:(c + 1) * G]):
            n = len(idx)
            if n:
                logits[idx] = out_c[:n, 2 * g:2 * g + 2]
    return logits, res


def kernel(**inputs) -> np.ndarray:
    logits, _ = _run(inputs, trace=False)
    return logits
